# revision 1
# baseline (speedup 1.0000x reference)
"""Trainium2 Bass kernel for nn_MlroleNode_64716567216639 (GAT message passing).

Math note: the reference model computes a dense NxN GATv2 attention but only
row 0 of the output (gat_out[0]) feeds the final MLP, so this kernel computes
just that row: e[j,h] = leaky(g_l[j] + g_r[0]) . w_attn, softmax over the 1024
source nodes, then a weighted sum of g_r values, followed by the 3-layer
type-define MLP over the 1023 ambiguous nodes.

Layout: features on partitions, nodes on the free axis (everything transposed
on host). The GAT row-0 computation is replicated on all 8 cores; the final
MLP is sharded 128 nodes per core.
"""
import numpy as np

H = 64
N_AMB = 1023
N = 1024
HEADS = 4
HID = 64
RT = 4
APT = 3
SLOPE = 0.2
NCORES = 8
SHARD = 128  # MLP nodes per core (8*128 = 1024 = N_AMB padded by 1)

_compiled = None  # (nc, static_in_maps_builder)


def _build():
    import concourse.tile as tile
    from concourse import bacc, mybir

    dt = mybir.dt.float32
    AF = mybir.ActivationFunctionType
    ALU = mybir.AluOpType
    AX = mybir.AxisListType

    nc = bacc.Bacc("TRN2", target_bir_lowering=False, debug=False,
                   enable_asserts=False, num_devices=NCORES)

    def din(name, shape):
        return nc.dram_tensor(name, shape, dt, kind="ExternalInput").ap()

    ambT_d = din("ambT", [H, N_AMB])
    hidc_d = din("hidc", [H, 1])
    ta_d = din("ta", [H, RT * APT])
    WselfT_d = din("WselfT", [H, H])
    WmLT_d = din("WmLT", [H, H])
    WmRT_d = din("WmRT", [H, H])
    WtT_d = din("WtT", [H, RT * H])
    btT_d = din("btT", [H, RT])
    bsc_d = din("bsc", [H, 1])
    bmc_d = din("bmc", [H, 1])
    WlT0_d = din("WlT0", [H, 128])
    WlT1_d = din("WlT1", [H, 128])
    WrT_d = din("WrT", [H, HEADS * HID])
    Wexp_d = din("Wexp", [128, 128])
    fold_d = din("fold", [128, H])
    Wd0a_aug_d = din("Wd0a_aug", [H + 1, 64])
    Wd0bT_d = din("Wd0bT", [H, 64])
    Wd1_aug_d = din("Wd1_aug", [65, 128])
    Wd2T_d = din("Wd2T", [128, RT])
    bd2c_d = din("bd2c", [RT, 1])
    mlp_d = din("mlp_cols", [H, SHARD])
    outT_d = nc.dram_tensor("outT", [RT, SHARD], dt, kind="ExternalOutput").ap()

    with tile.TileContext(nc) as tc:
        with tc.tile_pool(name="wp", bufs=1) as wp, \
             tc.tile_pool(name="sb", bufs=1) as sb, \
             tc.tile_pool(name="ps", bufs=1, space="PSUM") as ps:

            # ---- load inputs to SBUF ----
            def load(dram_ap, shape, tag):
                t = wp.tile(shape, dt, tag=tag)
                nc.sync.dma_start(t[:], dram_ap[:])
                return t

            ta_sb = load(ta_d, [H, RT * APT], "ta")
            WselfT = load(WselfT_d, [H, H], "WselfT")
            WmLT = load(WmLT_d, [H, H], "WmLT")
            WmRT = load(WmRT_d, [H, H], "WmRT")
            WtT = load(WtT_d, [H, RT * H], "WtT")
            btT = load(btT_d, [H, RT], "btT")
            bsc = load(bsc_d, [H, 1], "bsc")
            bmc = load(bmc_d, [H, 1], "bmc")
            hidc = load(hidc_d, [H, 1], "hidc")
            WlT = [load(WlT0_d, [H, 128], "WlT0"), load(WlT1_d, [H, 128], "WlT1")]
            WrT = load(WrT_d, [H, HEADS * HID], "WrT")
            Wexp = load(Wexp_d, [128, 128], "Wexp")
            fold = load(fold_d, [128, H], "fold")
            Wd0a_aug = load(Wd0a_aug_d, [H + 1, 64], "Wd0a")
            Wd0bT = load(Wd0bT_d, [H, 64], "Wd0b")
            Wd1_aug = load(Wd1_aug_d, [65, 128], "Wd1")
            Wd2T = load(Wd2T_d, [128, RT], "Wd2")
            bd2c = load(bd2c_d, [RT, 1], "bd2c")

            hT = wp.tile([H, N], dt, tag="hT")
            nc.sync.dma_start(hT[:, 1:N], ambT_d[:])
            mlp_aug = wp.tile([H + 1, SHARD], dt, tag="mlpa")
            nc.sync.dma_start(mlp_aug[0:H, :], mlp_d[:])
            nc.vector.memset(mlp_aug[H:H + 1, :], 1.0)
            # preload ACT tables (Exp/Sigmoid) off the critical softmax path
            warm = wp.tile([1, 4], dt, tag="warm")
            nc.vector.memset(warm[:], 0.0)
            warm_act = wp.tile([1, 4], dt, tag="warmact")
            nc.scalar.activation(warm_act[0:1, 0:1], warm[0:1, 0:1], AF.Exp)

            def leaky(out_ap, in_ap):
                # in_ap must be SBUF (stt can read at most one PSUM input)
                nc.vector.scalar_tensor_tensor(out=out_ap, in0=in_ap, scalar=SLOPE,
                                               in1=in_ap, op0=ALU.mult, op1=ALU.max)

            def leaky_psum(out_ap, psum_ap, scratch_ap):
                # leaky(x) = max(0.2*x, x) with x in PSUM: two DVE ops
                nc.vector.tensor_scalar_mul(scratch_ap, psum_ap, SLOPE)
                nc.vector.tensor_tensor(out_ap, scratch_ap, psum_ap, op=ALU.max)

            # ---- prologue: role-type routing + merge chain -> h1 [64,1] ----
            tsum = sb.tile([H, RT], dt, tag="tsum")
            nc.vector.reduce_sum(tsum[:], ta_sb[:].rearrange("p (t a) -> p t a", a=APT),
                                 axis=AX.X)
            tmean = sb.tile([H, RT], dt, tag="tmean")
            nc.vector.tensor_scalar_mul(tmean[:], tsum[:], 1.0 / APT)
            tmp_ps = ps.tile([H, RT], dt, tag="sp", bufs=1)
            for t in range(RT):
                nc.tensor.matmul(tmp_ps[:, t:t + 1], WtT[:, H * t:H * (t + 1)],
                                 tmean[:, t:t + 1], start=True, stop=True)
            tmpc = sb.tile([H, RT], dt, tag="tmpc")
            nc.vector.tensor_add(tmpc[:], tmp_ps[:], btT[:])
            C_ps = ps.tile([H, RT], dt, tag="sp", bufs=1)
            nc.tensor.matmul(C_ps[:], WmRT[:], tmpc[:], start=True, stop=True)
            C_sb = sb.tile([H, RT], dt, tag="C")
            nc.vector.tensor_scalar_add(C_sb[:], C_ps[:], bmc[:])

            h1_ps = ps.tile([H, 1], dt, tag="sp", bufs=1)
            nc.tensor.matmul(h1_ps[:], WselfT[:], hidc[:], start=True, stop=True)
            h1 = sb.tile([H, 1], dt, tag="h1", bufs=2)
            nc.vector.tensor_scalar_add(h1[:], h1_ps[:], bsc[:])
            for t in range(RT):
                hp = ps.tile([H, 1], dt, tag="sp", bufs=1)
                nc.tensor.matmul(hp[:], WmLT[:], h1[:], start=True, stop=True)
                u = sb.tile([H, 1], dt, tag="u", bufs=2)
                nc.vector.tensor_scalar_add(u[:], hp[:], C_sb[:, t:t + 1])
                h1n = sb.tile([H, 1], dt, tag="h1", bufs=2)
                leaky(h1n[:], u[:])
                h1 = h1n
            nc.vector.tensor_copy(hT[:, 0:1], h1[:])

            # ---- GAT row 0, two head-pair blocks ----
            h2_ps = ps.tile([H, 1], dt, tag="h2ps", bufs=1)
            for b in range(2):
                # g_r0 column for this head-pair block (attention query side)
                gr0_ps = ps.tile([128, 1], dt, tag="sp", bufs=1)
                nc.tensor.matmul(gr0_ps[:], WrT[:, 128 * b:128 * b + 128], h1[:],
                                 start=True, stop=True)
                gr0c = sb.tile([128, 1], dt, tag="gr0", bufs=2)
                nc.vector.tensor_copy(gr0c[:], gr0_ps[:])
                gl_ps = ps.tile([128, N], dt, tag="gle", bufs=2)
                for c in (0, 512):
                    nc.tensor.matmul(gl_ps[:, c:c + 512], WlT[b][:], hT[:, c:c + 512],
                                     start=True, stop=True)
                t_sb = sb.tile([128, N], dt, tag="t", bufs=2)
                u_sb = sb.tile([128, N], dt, tag="scr", bufs=2)
                nc.scalar.activation(u_sb[:], gl_ps[:], AF.Identity, bias=gr0c[:])
                leaky(t_sb[:], u_sb[:])
                gr_ps = ps.tile([128, N], dt, tag="gr", bufs=1)
                for c in (0, 512):
                    nc.tensor.matmul(gr_ps[:, c:c + 512],
                                     WrT[:, 128 * b:128 * b + 128],
                                     hT[:, c:c + 512], start=True, stop=True)
                e_ps = ps.tile([128, N], dt, tag="gle", bufs=2)
                for c in (0, 512):
                    nc.tensor.matmul(e_ps[:, c:c + 512], Wexp[:], t_sb[:, c:c + 512],
                                     start=True, stop=True)
                # softmax over the 1024 source nodes (per head, replicated x64).
                # logits are O(5) so no max subtraction is needed in fp32.
                pexp = sb.tile([128, N], dt, tag="pexp", bufs=2)
                ssum = sb.tile([128, 1], dt, tag="s", bufs=4)
                nc.scalar.activation(pexp[:], e_ps[:], AF.Exp, bias=0.0,
                                     accum_out=ssum[:])
                # weighted value sum over source nodes (fused mul + row-sum)
                scr = sb.tile([128, N], dt, tag="scr", bufs=2)
                att_u = sb.tile([128, 1], dt, tag="acc", bufs=4)
                nc.vector.scalar_tensor_tensor(
                    out=scr[:], in0=pexp[:], scalar=1.0, in1=gr_ps[:],
                    op0=ALU.mult, op1=ALU.mult, accum_out=att_u[:])
                rs = sb.tile([128, 1], dt, tag="s", bufs=4)
                nc.vector.reciprocal(rs[:], ssum[:])
                att_n = sb.tile([128, 1], dt, tag="acc", bufs=4)
                nc.vector.tensor_mul(att_n[:], att_u[:], rs[:])
                # fold heads: h2 += 0.25 * sum over the 2 heads in this block
                nc.tensor.matmul(h2_ps[:], fold[:], att_n[:], start=(b == 0),
                                 stop=(b == 1))

            h2 = sb.tile([H, 1], dt, tag="h2")
            nc.vector.tensor_copy(h2[:], h2_ps[:])

            # ---- final MLP on this core's 128-node shard ----
            c0_ps = ps.tile([H, 1], dt, tag="sp", bufs=1)
            nc.tensor.matmul(c0_ps[:], Wd0bT[:], h2[:], start=True, stop=True)
            c0col = sb.tile([H, 1], dt, tag="c0")
            nc.vector.tensor_copy(c0col[:], c0_ps[:])
            y0_ps = ps.tile([64, SHARD], dt, tag="sp", bufs=1)
            nc.tensor.matmul(y0_ps[:], Wd0a_aug[:], mlp_aug[:], start=True, stop=True)
            y0_aug = sb.tile([65, SHARD], dt, tag="y0")
            nc.vector.memset(y0_aug[64:65, :], 1.0)
            y0u = sb.tile([64, SHARD], dt, tag="yscr", bufs=2)
            nc.scalar.activation(y0u[:], y0_ps[:], AF.Identity, bias=c0col[:])
            leaky(y0_aug[0:64, :], y0u[:])
            y1_ps = ps.tile([128, SHARD], dt, tag="sp", bufs=1)
            nc.tensor.matmul(y1_ps[:], Wd1_aug[:], y0_aug[:], start=True, stop=True)
            y1 = sb.tile([128, SHARD], dt, tag="y1")
            y1scr = sb.tile([128, SHARD], dt, tag="yscr", bufs=2)
            leaky_psum(y1[:], y1_ps[:], y1scr[:])
            o_ps = ps.tile([RT, SHARD], dt, tag="sp", bufs=1)
            nc.tensor.matmul(o_ps[:], Wd2T[:], y1[:], start=True, stop=True)
            # sigmoid(z) = 1/(1+exp(-z)) using the already-loaded Exp table
            # (avoids a 1.3us Sigmoid ACT-table load on the critical path)
            o_e = sb.tile([RT, SHARD], dt, tag="oe")
            nc.scalar.activation(o_e[:], o_ps[:], AF.Exp, bias=bd2c[:], scale=-1.0)
            o_1p = sb.tile([RT, SHARD], dt, tag="o1p")
            nc.vector.tensor_scalar_add(o_1p[:], o_e[:], 1.0)
            o_sb = sb.tile([RT, SHARD], dt, tag="o")
            nc.vector.reciprocal(o_sb[:], o_1p[:])
            nc.sync.dma_start(outT_d[:], o_sb[:])

    nc.compile()
    return nc


def _prep_inputs(inputs):
    f32 = np.float32

    def c(a):
        return np.ascontiguousarray(a, dtype=f32)

    hidden = np.asarray(inputs["hidden"], f32)
    ambiguous = np.asarray(inputs["ambiguous"], f32)
    type_agents = np.asarray(inputs["type_agents"], f32)
    W_self = np.asarray(inputs["W_self"], f32)
    b_self = np.asarray(inputs["b_self"], f32)
    W_merge = np.asarray(inputs["W_merge"], f32)
    b_merge = np.asarray(inputs["b_merge"], f32)
    W_trans = np.asarray(inputs["W_trans"], f32)
    b_trans = np.asarray(inputs["b_trans"], f32)
    W_l = np.asarray(inputs["W_l"], f32)
    W_r = np.asarray(inputs["W_r"], f32)
    w_attn = np.asarray(inputs["w_attn"], f32)
    Wd0 = np.asarray(inputs["Wd0"], f32)
    bd0 = np.asarray(inputs["bd0"], f32)
    Wd1 = np.asarray(inputs["Wd1"], f32)
    bd1 = np.asarray(inputs["bd1"], f32)
    Wd2 = np.asarray(inputs["Wd2"], f32)
    bd2 = np.asarray(inputs["bd2"], f32)

    ambT = c(ambiguous.T)                                   # [64, 1023]
    WlT_full = c(W_l.T)                                     # [64, 256]
    Wexp = np.zeros((128, 128), f32)
    for hh in range(2):
        Wexp[hh * 64:(hh + 1) * 64, hh * 64:(hh + 1) * 64] = w_attn[:, None]
    fold = np.zeros((128, 64), f32)
    fold[np.arange(128), np.arange(128) % 64] = 0.25

    shared = {
        "ambT": ambT,
        "hidc": c(hidden.reshape(H, 1)),
        "ta": c(type_agents.reshape(RT * APT, H).T),
        "WselfT": c(W_self.T),
        "WmLT": c(W_merge[:, :H].T),
        "WmRT": c(W_merge[:, H:].T),
        "WtT": c(np.concatenate([W_trans[t].T for t in range(RT)], axis=1)),
        "btT": c(b_trans.T),
        "bsc": c(b_self.reshape(H, 1)),
        "bmc": c(b_merge.reshape(H, 1)),
        "WlT0": c(WlT_full[:, :128]),
        "WlT1": c(WlT_full[:, 128:]),
        "WrT": c(W_r.T),
        "Wexp": Wexp,
        "fold": fold,
        "Wd0a_aug": c(np.vstack([Wd0[:, :H].T, bd0[None, :]])),
        "Wd0bT": c(Wd0[:, H:].T),
        "Wd1_aug": c(np.vstack([Wd1.T, bd1[None, :]])),
        "Wd2T": c(Wd2.T),
        # negated: used as the bias of Exp(-z) inside the exp-based sigmoid
        "bd2c": c(-bd2.reshape(RT, 1)),
    }
    amb_pad = np.zeros((H, NCORES * SHARD), f32)
    amb_pad[:, :N_AMB] = ambT
    in_maps = []
    for cidx in range(NCORES):
        m = dict(shared)
        m["mlp_cols"] = c(amb_pad[:, cidx * SHARD:(cidx + 1) * SHARD])
        in_maps.append(m)
    return in_maps


def kernel(**inputs) -> np.ndarray:
    global _compiled
    if _compiled is None:
        _compiled = _build()
    nc = _compiled
    from concourse import bass_utils

    in_maps = _prep_inputs(inputs)
    res = bass_utils.run_bass_kernel_spmd(nc, in_maps, core_ids=list(range(NCORES)))
    out = np.empty((N_AMB, RT), np.float32)
    for cidx in range(NCORES):
        lo = cidx * SHARD
        hi = min(lo + SHARD, N_AMB)
        out[lo:hi, :] = res.results[cidx]["outT"][:, :hi - lo].T
    return out



# revision 7
# speedup vs baseline: 1.9121x; 1.9121x over previous
"""Trainium2 Bass kernel for nn_MlroleNode_64716567216639 (GAT message passing).

Math note: the reference computes a dense NxN GATv2 attention but only row 0
of the output feeds the final MLP, so this kernel computes just that row:
e[j,h] = leaky(g_l[j] + g_r[0]) . w_attn over the 1024 source nodes, softmax,
weighted sum of g_r values, then the 3-layer type-define MLP over the 1023
ambiguous nodes (sharded 128 nodes per core; GAT row-0 replicated).

Optimizations vs the naive version:
- All inputs packed into ONE bf16 blob + one tiny fp32 blob -> 3 dma_starts
  instead of 22 (each dma_start costs ~600ns serially on the Sync engine).
- 64-row weights ride the unused bottom partitions (64:128) of the blob; the
  matmuls that consume them run in the lower PE quadrant via
  tile_position=(64, .).
- bf16 matmuls: single PE pass (fp32 runs LOW_HIGH = 4 passes).
- leaky(x + bias) fused into one scalar-engine ACT (Prelu, alpha=0.2) reading
  straight from PSUM. Prelu lives in the same ACT table as Exp -> no table
  switches; Sigmoid's table is preloaded via a dummy ACT after the last Exp.
- softmax 1/sum via the single-op approximate reciprocal instead of the
  ~1.1us DVE reciprocal.
"""
import numpy as np

H = 64
N_AMB = 1023
N = 1024
HEADS = 4
RT = 4
APT = 3
SLOPE = 0.2
NCORES = 8
SHARD = 128

# bf16 blob column map (see _prep_inputs)
C_WL = 0        # top: W_l.T            [64, 256]
C_WR = 256      # top: W_r.T            [64, 256]
C_HT = 512      # top: hT (node 0 = h1 slot, zero), nodes j at col C_HT+j
C_WT = 0        # bottom: W_trans[t].T/3  [64, 256]
C_WSELF = 256   # bottom: W_self.T      [64, 64]
C_WML = 320     # bottom: W_merge[:, :64].T
C_WMR = 384     # bottom: W_merge[:, 64:].T
C_WD0B = 448    # bottom: Wd0[:, 64:].T
C_MLP = 512     # bottom: per-core mlp amb slice [64, 128]
C_WD0A = 640    # bottom: Wd0[:, :64].T
C_WD1 = 704     # bottom: Wd1.T         [64, 128]
C_TA = 832      # bottom: type agents   [64, 12]
C_BTT = 844     # bottom: b_trans.T     [64, 4]
C_HID = 848     # bottom: hidden.T      [64, 1]
C_WEXP = 1536   # full: block-diag w_attn  [128, 128]
C_FOLD = 1664   # full: 0.25 head-fold  [128, 64]
C_WD2 = 1728    # full: Wd2.T           [128, 4]
CB = 1732

_compiled = None


def _build():
    import concourse.tile as tile
    from concourse import bacc, mybir

    f32 = mybir.dt.float32
    bf = mybir.dt.bfloat16
    AF = mybir.ActivationFunctionType
    ALU = mybir.AluOpType
    AX = mybir.AxisListType

    nc = bacc.Bacc("TRN2", target_bir_lowering=False, debug=False,
                   enable_asserts=False, num_devices=NCORES)

    bfb_d = nc.dram_tensor("bfb", [128, CB], bf, kind="ExternalInput").ap()
    f32b_d = nc.dram_tensor("f32b", [128, 5], f32, kind="ExternalInput").ap()
    outT_d = nc.dram_tensor("outT", [RT, SHARD], f32, kind="ExternalOutput").ap()

    with nc.allow_low_precision("bf16 kernel, tolerance 2e-2"), \
         tile.TileContext(nc) as tc:
        with tc.tile_pool(name="wp", bufs=1) as wp, \
             tc.tile_pool(name="sb", bufs=1) as sb, \
             tc.tile_pool(name="ps", bufs=1, space="PSUM") as ps:

            B = wp.tile([128, CB], bf, tag="bfb")
            F = wp.tile([128, 5], f32, tag="f32b")
            nc.sync.dma_start(B[:, 0:1024], bfb_d[:, 0:1024])
            nc.sync.dma_start(F[:], f32b_d[:])
            nc.sync.dma_start(B[:, 1024:CB], bfb_d[:, 1024:CB])

            bsc = F[64:128, 0:1]
            bmc = F[64:128, 1:2]
            bd0c = F[0:64, 2:3]
            bd1c = F[0:128, 3:4]
            bd2c = F[0:4, 4:5]

            # preload the Exp table off the critical path (Prelu/Identity/Exp
            # all live in the same table set)
            warm = wp.tile([1, 4], f32, tag="warm")
            nc.vector.memset(warm[:], 0.0)
            warm_act = wp.tile([1, 4], f32, tag="warmact")
            nc.scalar.activation(warm_act[0:1, 0:1], warm[0:1, 0:1], AF.Exp)

            # single-bank PSUM arena for all small matmul outputs (PSUM
            # allocation is bank-granular; separate tags would need a bank
            # each). Regions are disjoint columns of one [128, 512] bank.
            arena = ps.tile([128, 512], f32, tag="sp", bufs=1)
            y0_ps = arena[0:64, 0:SHARD]
            y1_ps = arena[0:128, 128:256]
            o_ps = arena[0:4, 256:384]
            h2p_ps = arena[0:64, 384:386]
            c0_ps = arena[0:64, 388:389]
            tmp_ps = arena[64:128, 392:396]
            C_ps = arena[64:128, 396:400]
            h1_ps = arena[64:128, 400:401]

            # ---- prologue: role-type routing + merge chain (bottom half) ----
            tsum = sb.tile([128, RT], bf, tag="tsum")
            nc.vector.reduce_sum(
                tsum[64:128, :],
                B[64:128, C_TA:C_TA + RT * APT].rearrange("p (t a) -> p t a", a=APT),
                axis=AX.X)
            for t in range(RT):
                nc.tensor.matmul(tmp_ps[:, t:t + 1],
                                 B[64:128, C_WT + H * t:C_WT + H * (t + 1)],
                                 tsum[64:128, t:t + 1], start=True, stop=True,
                                 tile_position=(64, 64))
            tmpc = sb.tile([128, RT], bf, tag="tmpc")
            nc.vector.tensor_tensor(tmpc[64:128, :], tmp_ps[:],
                                    B[64:128, C_BTT:C_BTT + RT], op=ALU.add)
            nc.tensor.matmul(C_ps[:], B[64:128, C_WMR:C_WMR + H],
                             tmpc[64:128, :], start=True, stop=True,
                             tile_position=(64, 64))
            C_sb = sb.tile([128, RT], f32, tag="C")
            nc.vector.tensor_scalar_add(C_sb[64:128, :], C_ps[:], bmc)

            # h1 chain: h1 = W_self @ hidden + b_self, then 4x leaky-merge
            h1t = sb.tile([128, RT + 1], bf, tag="h1t")
            nc.tensor.matmul(h1_ps[:], B[64:128, C_WSELF:C_WSELF + H],
                             B[64:128, C_HID:C_HID + 1], start=True, stop=True,
                             tile_position=(64, 64))
            nc.scalar.activation(h1t[64:128, 0:1], h1_ps[:], AF.Identity,
                                 bias=bsc)
            for t in range(RT):
                hp = arena[64:128, 402 + 2 * (t % 2):403 + 2 * (t % 2)]
                nc.tensor.matmul(hp, B[64:128, C_WML:C_WML + H],
                                 h1t[64:128, t:t + 1], start=True, stop=True,
                                 tile_position=(64, 64))
                if t < RT - 1:
                    nc.scalar.activation(h1t[64:128, t + 1:t + 2], hp,
                                         AF.Prelu, bias=C_sb[64:128, t:t + 1],
                                         alpha=SLOPE)
                else:
                    # final h1 -> node-0 column of hT (top half)
                    nc.scalar.activation(B[0:64, C_HT:C_HT + 1], hp,
                                         AF.Prelu, bias=C_sb[64:128, t:t + 1],
                                         alpha=SLOPE)

            # attention query columns g_r[0] per head-pair block
            gr0c = sb.tile([128, 2], f32, tag="gr0c")
            for b in range(2):
                gr0_ps = arena[0:128, 408 + 2 * b:409 + 2 * b]
                nc.tensor.matmul(gr0_ps, B[0:64, C_WR + 128 * b:C_WR + 128 * b + 128],
                                 B[0:64, C_HT:C_HT + 1], start=True, stop=True)
                nc.vector.tensor_copy(gr0c[:, b:b + 1], gr0_ps)

            # ---- first MLP matmul on this core's shard (h2-independent) ----
            nc.tensor.matmul(y0_ps, B[64:128, C_WD0A:C_WD0A + H],
                             B[64:128, C_MLP:C_MLP + SHARD], start=True, stop=True,
                             tile_position=(64, 0))

            # ---- GAT row 0: 2 head-pair blocks x 2 column chunks of 512 ----
            ssum4 = sb.tile([128, 4], f32, tag="ssum4")
            att4 = sb.tile([128, 4], f32, tag="att4")
            pexp_last = None
            for b in range(2):
                for c in range(2):
                    u = 2 * b + c
                    cols = slice(C_HT + 512 * c, C_HT + 512 * (c + 1))
                    gl_ps = ps.tile([128, 512], f32, tag="ge", bufs=3)
                    nc.tensor.matmul(gl_ps[:],
                                     B[0:64, C_WL + 128 * b:C_WL + 128 * b + 128],
                                     B[0:64, cols], start=True, stop=True)
                    t_sb = sb.tile([128, 512], bf, tag="t", bufs=2)
                    nc.scalar.activation(t_sb[:], gl_ps[:], AF.Prelu,
                                         bias=gr0c[:, b:b + 1], alpha=SLOPE)
                    e_ps = ps.tile([128, 512], f32, tag="ge", bufs=3)
                    nc.tensor.matmul(e_ps[:], B[:, C_WEXP:C_WEXP + 128], t_sb[:],
                                     start=True, stop=True)
                    gr_ps = ps.tile([128, 512], f32, tag="gr", bufs=3)
                    nc.tensor.matmul(gr_ps[:],
                                     B[0:64, C_WR + 128 * b:C_WR + 128 * b + 128],
                                     B[0:64, cols], start=True, stop=True)
                    pexp = sb.tile([128, 512], f32, tag="pexp", bufs=2)
                    nc.scalar.activation(pexp[:], e_ps[:], AF.Exp, bias=0.0,
                                         accum_out=ssum4[:, u:u + 1])
                    scr = sb.tile([128, 512], bf, tag="scr", bufs=2)
                    nc.vector.scalar_tensor_tensor(
                        out=scr[:], in0=pexp[:], scalar=1.0, in1=gr_ps[:],
                        op0=ALU.mult, op1=ALU.mult, accum_out=att4[:, u:u + 1])
                    pexp_last = pexp

            # preload the Sigmoid table while the MLP matmuls run. Reading a
            # row of ssum4 makes this depend on ALL four Exp accumulators, so
            # the table switch is ordered strictly after the last Exp (the
            # same table also holds Prelu, so later Prelu ACTs don't reload).
            warm_sig = wp.tile([1, 4], f32, tag="warmsig")
            nc.scalar.activation(warm_sig[0:1, 0:4], ssum4[0:1, 0:4], AF.Sigmoid)

            # combine chunks, normalize, fold heads
            ssum2 = sb.tile([128, 2], f32, tag="ssum2")
            att2 = sb.tile([128, 2], f32, tag="att2")
            for b in range(2):
                nc.vector.tensor_tensor(ssum2[:, b:b + 1], ssum4[:, 2 * b:2 * b + 1],
                                        ssum4[:, 2 * b + 1:2 * b + 2], op=ALU.add)
                nc.vector.tensor_tensor(att2[:, b:b + 1], att4[:, 2 * b:2 * b + 1],
                                        att4[:, 2 * b + 1:2 * b + 2], op=ALU.add)
            rs2 = sb.tile([128, 2], f32, tag="rs2")
            nc.vector.reciprocal_approx_fast(rs2[:], ssum2[:])
            attn2 = sb.tile([128, 2], bf, tag="attn2")
            nc.vector.tensor_tensor(attn2[:], att2[:], rs2[:], op=ALU.mult)
            h2_ps = h2p_ps[:, 0:1]
            for b in range(2):
                nc.tensor.matmul(h2_ps, B[:, C_FOLD:C_FOLD + H], attn2[:, b:b + 1],
                                 start=(b == 0), stop=(b == 1))
            h2b = sb.tile([128, 1], bf, tag="h2b")
            nc.vector.tensor_copy(h2b[64:128, :], h2_ps)

            # ---- final MLP (gated on h2) ----
            nc.tensor.matmul(c0_ps, B[64:128, C_WD0B:C_WD0B + H],
                             h2b[64:128, :], start=True, stop=True,
                             tile_position=(64, 0))
            c0col = sb.tile([64, 1], f32, tag="c0col")
            nc.vector.tensor_scalar_add(c0col[:], c0_ps, bd0c)
            y0b = sb.tile([128, SHARD], bf, tag="y0b")
            nc.scalar.activation(y0b[64:128, :], y0_ps, AF.Prelu, bias=c0col[:],
                                 alpha=SLOPE)
            nc.tensor.matmul(y1_ps, B[64:128, C_WD1:C_WD1 + SHARD],
                             y0b[64:128, :], start=True, stop=True,
                             tile_position=(64, 0))
            y1f = sb.tile([128, SHARD], bf, tag="y1f")
            nc.scalar.activation(y1f[:], y1_ps, AF.Prelu, bias=bd1c, alpha=SLOPE)
            nc.tensor.matmul(o_ps, B[:, C_WD2:C_WD2 + RT], y1f[:],
                             start=True, stop=True)
            o_sb = sb.tile([RT, SHARD], f32, tag="osb")
            nc.scalar.activation(o_sb[:], o_ps, AF.Sigmoid, bias=bd2c)
            nc.sync.dma_start(outT_d[:], o_sb[:])

    nc.compile()
    return nc


def _prep_inputs(inputs):
    import ml_dtypes
    bf16 = ml_dtypes.bfloat16
    f32 = np.float32

    hidden = np.asarray(inputs["hidden"], f32)
    ambiguous = np.asarray(inputs["ambiguous"], f32)
    type_agents = np.asarray(inputs["type_agents"], f32)
    W_self = np.asarray(inputs["W_self"], f32)
    b_self = np.asarray(inputs["b_self"], f32)
    W_merge = np.asarray(inputs["W_merge"], f32)
    b_merge = np.asarray(inputs["b_merge"], f32)
    W_trans = np.asarray(inputs["W_trans"], f32)
    b_trans = np.asarray(inputs["b_trans"], f32)
    W_l = np.asarray(inputs["W_l"], f32)
    W_r = np.asarray(inputs["W_r"], f32)
    w_attn = np.asarray(inputs["w_attn"], f32)
    Wd0 = np.asarray(inputs["Wd0"], f32)
    bd0 = np.asarray(inputs["bd0"], f32)
    Wd1 = np.asarray(inputs["Wd1"], f32)
    bd1 = np.asarray(inputs["bd1"], f32)
    Wd2 = np.asarray(inputs["Wd2"], f32)
    bd2 = np.asarray(inputs["bd2"], f32)

    base = np.zeros((128, CB), f32)
    top = base[0:64]
    bot = base[64:128]
    top[:, C_WL:C_WL + 256] = W_l.T
    top[:, C_WR:C_WR + 256] = W_r.T
    top[:, C_HT + 1:C_HT + N] = ambiguous.T
    bot[:, C_WT:C_WT + 256] = np.concatenate(
        [W_trans[t].T for t in range(RT)], axis=1) / APT
    bot[:, C_WSELF:C_WSELF + H] = W_self.T
    bot[:, C_WML:C_WML + H] = W_merge[:, :H].T
    bot[:, C_WMR:C_WMR + H] = W_merge[:, H:].T
    bot[:, C_WD0B:C_WD0B + H] = Wd0[:, H:].T
    bot[:, C_WD0A:C_WD0A + H] = Wd0[:, :H].T
    bot[:, C_WD1:C_WD1 + SHARD] = Wd1.T
    bot[:, C_TA:C_TA + RT * APT] = type_agents.reshape(RT * APT, H).T
    bot[:, C_BTT:C_BTT + RT] = b_trans.T
    bot[:, C_HID:C_HID + 1] = hidden.T
    wexp = np.zeros((128, 128), f32)
    for hh in range(2):
        wexp[hh * 64:(hh + 1) * 64, hh * 64:(hh + 1) * 64] = w_attn[:, None]
    base[:, C_WEXP:C_WEXP + 128] = wexp
    fold = np.zeros((128, H), f32)
    fold[np.arange(128), np.arange(128) % H] = 0.25
    base[:, C_FOLD:C_FOLD + H] = fold
    base[:, C_WD2:C_WD2 + RT] = Wd2.T

    f32b = np.zeros((128, 5), f32)
    f32b[64:128, 0] = b_self
    f32b[64:128, 1] = b_merge
    f32b[0:64, 2] = bd0
    f32b[0:128, 3] = bd1
    f32b[0:RT, 4] = bd2

    amb_pad = np.zeros((H, NCORES * SHARD), f32)
    amb_pad[:, :N_AMB] = ambiguous.T
    in_maps = []
    for cidx in range(NCORES):
        blob = base.copy()
        blob[64:128, C_MLP:C_MLP + SHARD] = \
            amb_pad[:, cidx * SHARD:(cidx + 1) * SHARD]
        in_maps.append({"bfb": blob.astype(bf16), "f32b": f32b})
    return in_maps


def kernel(**inputs) -> np.ndarray:
    global _compiled
    if _compiled is None:
        _compiled = _build()
    nc = _compiled
    from concourse import bass_utils

    in_maps = _prep_inputs(inputs)
    res = bass_utils.run_bass_kernel_spmd(nc, in_maps, core_ids=list(range(NCORES)))
    out = np.empty((N_AMB, RT), np.float32)
    for cidx in range(NCORES):
        lo = cidx * SHARD
        hi = min(lo + SHARD, N_AMB)
        out[lo:hi, :] = res.results[cidx]["outT"][:, :hi - lo].T
    return out


# revision 11
# speedup vs baseline: 1.9946x; 1.0431x over previous
"""Trainium2 Bass kernel for nn_MlroleNode_64716567216639 (GAT message passing).

Math note: the reference computes a dense NxN GATv2 attention but only row 0
of the output feeds the final MLP, so this kernel computes just that row:
e[j,h] = leaky(g_l[j] + g_r[0]) . w_attn over the 1024 source nodes, softmax,
weighted sum of g_r values, then the 3-layer type-define MLP over the 1023
ambiguous nodes (sharded 128 nodes per core; GAT row-0 replicated).

Optimizations vs the naive version:
- All inputs packed into ONE bf16 blob + one tiny fp32 blob -> 3 dma_starts
  instead of 22 (each dma_start costs ~600ns serially on the Sync engine).
- 64-row weights ride the unused bottom partitions (64:128) of the blob; the
  matmuls that consume them run in the lower PE quadrant via
  tile_position=(64, .).
- bf16 matmuls: single PE pass (fp32 runs LOW_HIGH = 4 passes).
- leaky(x + bias) fused into one scalar-engine ACT (Prelu, alpha=0.2) reading
  straight from PSUM. Prelu lives in the same ACT table as Exp -> no table
  switches; Sigmoid's table is preloaded via a dummy ACT after the last Exp.
- softmax 1/sum via the single-op approximate reciprocal instead of the
  ~1.1us DVE reciprocal.
"""
import numpy as np

H = 64
N_AMB = 1023
N = 1024
HEADS = 4
RT = 4
APT = 3
SLOPE = 0.2
NCORES = 8
SHARD = 128

# bf16 blob column map (see _prep_inputs)
C_WL = 0        # top: W_l.T            [64, 256]
C_WR = 256      # top: W_r.T            [64, 256]
C_HT = 512      # top: hT (node 0 = h1 slot, zero), nodes j at col C_HT+j
C_WT = 0        # bottom: W_trans[t].T/3  [64, 256]
C_WSELF = 256   # bottom: W_self.T      [64, 64]
C_WML = 320     # bottom: W_merge[:, :64].T
C_WMR = 384     # bottom: W_merge[:, 64:].T
C_TA = 448      # bottom: type agents   [64, 12]
C_BTT = 460     # bottom: b_trans.T     [64, 4]
C_HID = 464     # bottom: hidden.T      [64, 1]
C_WD0B = 468    # bottom: Wd0[:, 64:].T
C_MLP = 532     # bottom: per-core mlp amb slice [64, 128]
C_WD0A = 660    # bottom: Wd0[:, :64].T
C_WD1 = 724     # bottom: Wd1.T         [64, 128]
C_WEXP = 1536   # full: block-diag w_attn  [128, 128]
C_FOLD = 1664   # full: 0.25 head-fold  [128, 64]
C_WD2 = 1728    # full: Wd2.T           [128, 4]
CB = 1732

_compiled = None


def _build():
    import concourse.tile as tile
    from concourse import bacc, mybir

    f32 = mybir.dt.float32
    bf = mybir.dt.bfloat16
    AF = mybir.ActivationFunctionType
    ALU = mybir.AluOpType
    AX = mybir.AxisListType

    nc = bacc.Bacc("TRN2", target_bir_lowering=False, debug=False,
                   enable_asserts=False, num_devices=NCORES)

    bfb_d = nc.dram_tensor("bfb", [128, CB], bf, kind="ExternalInput").ap()
    f32b_d = nc.dram_tensor("f32b", [128, 5], f32, kind="ExternalInput").ap()
    outT_d = nc.dram_tensor("outT", [RT, SHARD], f32, kind="ExternalOutput").ap()

    with nc.allow_low_precision("bf16 kernel, tolerance 2e-2"), \
         tile.TileContext(nc) as tc:
        with tc.tile_pool(name="wp", bufs=1) as wp, \
             tc.tile_pool(name="sb", bufs=1) as sb, \
             tc.tile_pool(name="ps", bufs=1, space="PSUM") as ps:

            B = wp.tile([128, CB], bf, tag="bfb")
            F = wp.tile([128, 5], f32, tag="f32b")
            # ordered by when consumers need the data: prologue weights
            # first (the serial merge chain is the head of the critical
            # path), then biases, the full top half (W_l/W_r + hT), the
            # full-height tail (Wexp/fold/Wd2), and the MLP weights last.
            # Bottom cols 852:1536 are zeros and never transferred.
            nc.sync.dma_start(B[64:128, 0:C_WD0B], bfb_d[64:128, 0:C_WD0B])
            nc.sync.dma_start(F[:], f32b_d[:])
            nc.sync.dma_start(B[0:64, 0:C_WEXP], bfb_d[0:64, 0:C_WEXP])
            nc.sync.dma_start(B[:, C_WEXP:CB], bfb_d[:, C_WEXP:CB])
            nc.sync.dma_start(B[64:128, C_WD0B:852], bfb_d[64:128, C_WD0B:852])

            bsc = F[64:128, 0:1]
            bmc = F[64:128, 1:2]
            bd0c = F[0:64, 2:3]
            bd1c = F[0:128, 3:4]
            bd2c = F[0:4, 4:5]

            # preload the Exp table off the critical path (Prelu/Identity/Exp
            # all live in the same table set)
            warm = wp.tile([1, 4], f32, tag="warm")
            nc.vector.memset(warm[:], 0.0)
            warm_act = wp.tile([1, 4], f32, tag="warmact")
            nc.scalar.activation(warm_act[0:1, 0:1], warm[0:1, 0:1], AF.Exp)

            # single-bank PSUM arena for all small matmul outputs (PSUM
            # allocation is bank-granular; separate tags would need a bank
            # each). Regions are disjoint columns of one [128, 512] bank.
            arena = ps.tile([128, 512], f32, tag="sp", bufs=1)
            y0_ps = arena[0:64, 0:SHARD]
            y1_ps = arena[0:128, 128:256]
            o_ps = arena[0:4, 256:384]
            h2p_ps = arena[0:64, 384:386]
            c0_ps = arena[0:64, 388:389]
            tmp_ps = arena[64:128, 392:396]
            C_ps = arena[64:128, 396:400]
            h1_ps = arena[64:128, 400:401]

            # ---- prologue: role-type routing + merge chain (bottom half) ----
            tsum = sb.tile([128, RT], bf, tag="tsum")
            nc.vector.reduce_sum(
                tsum[64:128, :],
                B[64:128, C_TA:C_TA + RT * APT].rearrange("p (t a) -> p t a", a=APT),
                axis=AX.X)
            for t in range(RT):
                nc.tensor.matmul(tmp_ps[:, t:t + 1],
                                 B[64:128, C_WT + H * t:C_WT + H * (t + 1)],
                                 tsum[64:128, t:t + 1], start=True, stop=True,
                                 tile_position=(64, 64))
            tmpc = sb.tile([128, RT], bf, tag="tmpc")
            nc.vector.tensor_tensor(tmpc[64:128, :], tmp_ps[:],
                                    B[64:128, C_BTT:C_BTT + RT], op=ALU.add)
            nc.tensor.matmul(C_ps[:], B[64:128, C_WMR:C_WMR + H],
                             tmpc[64:128, :], start=True, stop=True,
                             tile_position=(64, 64))
            C_sb = sb.tile([128, RT], f32, tag="C")
            nc.vector.tensor_scalar_add(C_sb[64:128, :], C_ps[:], bmc)

            # h1 chain: h1 = W_self @ hidden + b_self, then 4x leaky-merge
            h1t = sb.tile([128, RT + 1], bf, tag="h1t")
            nc.tensor.matmul(h1_ps[:], B[64:128, C_WSELF:C_WSELF + H],
                             B[64:128, C_HID:C_HID + 1], start=True, stop=True,
                             tile_position=(64, 64))
            nc.scalar.activation(h1t[64:128, 0:1], h1_ps[:], AF.Identity,
                                 bias=bsc)
            for t in range(RT):
                hp = arena[64:128, 402 + 2 * (t % 2):403 + 2 * (t % 2)]
                nc.tensor.matmul(hp, B[64:128, C_WML:C_WML + H],
                                 h1t[64:128, t:t + 1], start=True, stop=True,
                                 tile_position=(64, 64))
                if t < RT - 1:
                    nc.scalar.activation(h1t[64:128, t + 1:t + 2], hp,
                                         AF.Prelu, bias=C_sb[64:128, t:t + 1],
                                         alpha=SLOPE)
                else:
                    # final h1 -> node-0 column of hT (top half)
                    nc.scalar.activation(B[0:64, C_HT:C_HT + 1], hp,
                                         AF.Prelu, bias=C_sb[64:128, t:t + 1],
                                         alpha=SLOPE)

            # attention query columns g_r[0] per head-pair block
            gr0c = sb.tile([128, 2], f32, tag="gr0c")
            for b in range(2):
                gr0_ps = arena[0:128, 408 + 2 * b:409 + 2 * b]
                nc.tensor.matmul(gr0_ps, B[0:64, C_WR + 128 * b:C_WR + 128 * b + 128],
                                 B[0:64, C_HT:C_HT + 1], start=True, stop=True)
                nc.vector.tensor_copy(gr0c[:, b:b + 1], gr0_ps)

            # ---- first MLP matmul on this core's shard (h2-independent) ----
            nc.tensor.matmul(y0_ps, B[64:128, C_WD0A:C_WD0A + H],
                             B[64:128, C_MLP:C_MLP + SHARD], start=True, stop=True,
                             tile_position=(64, 0))

            # ---- GAT row 0: 2 head-pair blocks x 2 column chunks of 512 ----
            ssum4 = sb.tile([128, 4], f32, tag="ssum4")
            att4 = sb.tile([128, 4], f32, tag="att4")
            pexp_last = None
            for b in range(2):
                for c in range(2):
                    u = 2 * b + c
                    cols = slice(C_HT + 512 * c, C_HT + 512 * (c + 1))
                    gl_ps = ps.tile([128, 512], f32, tag="ge", bufs=3)
                    nc.tensor.matmul(gl_ps[:],
                                     B[0:64, C_WL + 128 * b:C_WL + 128 * b + 128],
                                     B[0:64, cols], start=True, stop=True)
                    t_sb = sb.tile([128, 512], bf, tag="t", bufs=2)
                    nc.scalar.activation(t_sb[:], gl_ps[:], AF.Prelu,
                                         bias=gr0c[:, b:b + 1], alpha=SLOPE)
                    e_ps = ps.tile([128, 512], f32, tag="ge", bufs=3)
                    nc.tensor.matmul(e_ps[:], B[:, C_WEXP:C_WEXP + 128], t_sb[:],
                                     start=True, stop=True)
                    gr_ps = ps.tile([128, 512], f32, tag="gr", bufs=3)
                    nc.tensor.matmul(gr_ps[:],
                                     B[0:64, C_WR + 128 * b:C_WR + 128 * b + 128],
                                     B[0:64, cols], start=True, stop=True)
                    pexp = sb.tile([128, 512], bf, tag="pexp", bufs=2)
                    nc.scalar.activation(pexp[:], e_ps[:], AF.Exp, bias=0.0)
                    # softmax denominator on DVE; keeps the scalar engine
                    # (the pipeline bottleneck) free for Prelu/Exp only
                    nc.vector.reduce_sum(ssum4[:, u:u + 1], pexp[:], axis=AX.X)
                    scr = sb.tile([128, 512], bf, tag="scr", bufs=2)
                    nc.vector.scalar_tensor_tensor(
                        out=scr[:], in0=pexp[:], scalar=1.0, in1=gr_ps[:],
                        op0=ALU.mult, op1=ALU.mult, accum_out=att4[:, u:u + 1])
                    pexp_last = pexp

            # preload the Sigmoid table while the MLP matmuls run. Reading a
            # row of ssum4 makes this depend on ALL four Exp accumulators, so
            # the table switch is ordered strictly after the last Exp (the
            # same table also holds Prelu, so later Prelu ACTs don't reload).
            warm_sig = wp.tile([1, 4], f32, tag="warmsig")
            nc.scalar.activation(warm_sig[0:1, 0:4], ssum4[0:1, 0:4], AF.Sigmoid)

            # combine chunks, normalize, fold heads
            ssum2 = sb.tile([128, 2], f32, tag="ssum2")
            att2 = sb.tile([128, 2], f32, tag="att2")
            for b in range(2):
                nc.gpsimd.tensor_tensor(ssum2[:, b:b + 1], ssum4[:, 2 * b:2 * b + 1],
                                        ssum4[:, 2 * b + 1:2 * b + 2], op=ALU.add)
                nc.vector.tensor_tensor(att2[:, b:b + 1], att4[:, 2 * b:2 * b + 1],
                                        att4[:, 2 * b + 1:2 * b + 2], op=ALU.add)
            rs2 = sb.tile([128, 2], f32, tag="rs2")
            nc.vector.reciprocal_approx_fast(rs2[:], ssum2[:])
            attn2 = sb.tile([128, 2], bf, tag="attn2")
            nc.vector.tensor_tensor(attn2[:], att2[:], rs2[:], op=ALU.mult)
            h2_ps = h2p_ps[:, 0:1]
            for b in range(2):
                nc.tensor.matmul(h2_ps, B[:, C_FOLD:C_FOLD + H], attn2[:, b:b + 1],
                                 start=(b == 0), stop=(b == 1))
            h2b = sb.tile([128, 1], bf, tag="h2b")
            nc.vector.tensor_copy(h2b[64:128, :], h2_ps)

            # ---- final MLP (gated on h2) ----
            nc.tensor.matmul(c0_ps, B[64:128, C_WD0B:C_WD0B + H],
                             h2b[64:128, :], start=True, stop=True,
                             tile_position=(64, 0))
            c0col = sb.tile([64, 1], f32, tag="c0col")
            nc.vector.tensor_scalar_add(c0col[:], c0_ps, bd0c)
            y0b = sb.tile([128, SHARD], bf, tag="y0b")
            nc.scalar.activation(y0b[64:128, :], y0_ps, AF.Prelu, bias=c0col[:],
                                 alpha=SLOPE)
            nc.tensor.matmul(y1_ps, B[64:128, C_WD1:C_WD1 + SHARD],
                             y0b[64:128, :], start=True, stop=True,
                             tile_position=(64, 0))
            y1f = sb.tile([128, SHARD], bf, tag="y1f")
            nc.scalar.activation(y1f[:], y1_ps, AF.Prelu, bias=bd1c, alpha=SLOPE)
            nc.tensor.matmul(o_ps, B[:, C_WD2:C_WD2 + RT], y1f[:],
                             start=True, stop=True)
            o_sb = sb.tile([RT, SHARD], f32, tag="osb")
            nc.scalar.activation(o_sb[:], o_ps, AF.Sigmoid, bias=bd2c)
            nc.sync.dma_start(outT_d[:], o_sb[:])

    nc.compile()
    return nc


def _prep_inputs(inputs):
    import ml_dtypes
    bf16 = ml_dtypes.bfloat16
    f32 = np.float32

    hidden = np.asarray(inputs["hidden"], f32)
    ambiguous = np.asarray(inputs["ambiguous"], f32)
    type_agents = np.asarray(inputs["type_agents"], f32)
    W_self = np.asarray(inputs["W_self"], f32)
    b_self = np.asarray(inputs["b_self"], f32)
    W_merge = np.asarray(inputs["W_merge"], f32)
    b_merge = np.asarray(inputs["b_merge"], f32)
    W_trans = np.asarray(inputs["W_trans"], f32)
    b_trans = np.asarray(inputs["b_trans"], f32)
    W_l = np.asarray(inputs["W_l"], f32)
    W_r = np.asarray(inputs["W_r"], f32)
    w_attn = np.asarray(inputs["w_attn"], f32)
    Wd0 = np.asarray(inputs["Wd0"], f32)
    bd0 = np.asarray(inputs["bd0"], f32)
    Wd1 = np.asarray(inputs["Wd1"], f32)
    bd1 = np.asarray(inputs["bd1"], f32)
    Wd2 = np.asarray(inputs["Wd2"], f32)
    bd2 = np.asarray(inputs["bd2"], f32)

    base = np.zeros((128, CB), f32)
    top = base[0:64]
    bot = base[64:128]
    top[:, C_WL:C_WL + 256] = W_l.T
    top[:, C_WR:C_WR + 256] = W_r.T
    top[:, C_HT + 1:C_HT + N] = ambiguous.T
    bot[:, C_WT:C_WT + 256] = np.concatenate(
        [W_trans[t].T for t in range(RT)], axis=1) / APT
    bot[:, C_WSELF:C_WSELF + H] = W_self.T
    bot[:, C_WML:C_WML + H] = W_merge[:, :H].T
    bot[:, C_WMR:C_WMR + H] = W_merge[:, H:].T
    bot[:, C_WD0B:C_WD0B + H] = Wd0[:, H:].T
    bot[:, C_WD0A:C_WD0A + H] = Wd0[:, :H].T
    bot[:, C_WD1:C_WD1 + SHARD] = Wd1.T
    bot[:, C_TA:C_TA + RT * APT] = type_agents.reshape(RT * APT, H).T
    bot[:, C_BTT:C_BTT + RT] = b_trans.T
    bot[:, C_HID:C_HID + 1] = hidden.T
    wexp = np.zeros((128, 128), f32)
    for hh in range(2):
        wexp[hh * 64:(hh + 1) * 64, hh * 64:(hh + 1) * 64] = w_attn[:, None]
    base[:, C_WEXP:C_WEXP + 128] = wexp
    fold = np.zeros((128, H), f32)
    fold[np.arange(128), np.arange(128) % H] = 0.25
    base[:, C_FOLD:C_FOLD + H] = fold
    base[:, C_WD2:C_WD2 + RT] = Wd2.T

    f32b = np.zeros((128, 5), f32)
    f32b[64:128, 0] = b_self
    f32b[64:128, 1] = b_merge
    f32b[0:64, 2] = bd0
    f32b[0:128, 3] = bd1
    f32b[0:RT, 4] = bd2

    amb_pad = np.zeros((H, NCORES * SHARD), f32)
    amb_pad[:, :N_AMB] = ambiguous.T
    in_maps = []
    for cidx in range(NCORES):
        blob = base.copy()
        blob[64:128, C_MLP:C_MLP + SHARD] = \
            amb_pad[:, cidx * SHARD:(cidx + 1) * SHARD]
        in_maps.append({"bfb": blob.astype(bf16), "f32b": f32b})
    return in_maps


def kernel(**inputs) -> np.ndarray:
    global _compiled
    if _compiled is None:
        _compiled = _build()
    nc = _compiled
    from concourse import bass_utils

    in_maps = _prep_inputs(inputs)
    res = bass_utils.run_bass_kernel_spmd(nc, in_maps, core_ids=list(range(NCORES)))
    out = np.empty((N_AMB, RT), np.float32)
    for cidx in range(NCORES):
        lo = cidx * SHARD
        hi = min(lo + SHARD, N_AMB)
        out[lo:hi, :] = res.results[cidx]["outT"][:, :hi - lo].T
    return out


# revision 13
# speedup vs baseline: 2.1578x; 1.0819x over previous
"""Trainium2 Bass kernel for nn_MlroleNode_64716567216639 (GAT message passing).

Math note: the reference computes a dense NxN GATv2 attention but only row 0
of the output feeds the final MLP, so this kernel computes just that row:
e[j,h] = leaky(g_l[j] + g_r[0]) . w_attn over the 1024 source nodes, softmax,
weighted sum of g_r values, then the 3-layer type-define MLP over the 1023
ambiguous nodes (sharded 128 nodes per core; GAT row-0 replicated).

Optimizations vs the naive version:
- All inputs packed into ONE bf16 blob + one tiny fp32 blob -> 3 dma_starts
  instead of 22 (each dma_start costs ~600ns serially on the Sync engine).
- 64-row weights ride the unused bottom partitions (64:128) of the blob; the
  matmuls that consume them run in the lower PE quadrant via
  tile_position=(64, .).
- bf16 matmuls: single PE pass (fp32 runs LOW_HIGH = 4 passes).
- leaky(x + bias) fused into one scalar-engine ACT (Prelu, alpha=0.2) reading
  straight from PSUM. Prelu lives in the same ACT table as Exp -> no table
  switches; Sigmoid's table is preloaded via a dummy ACT after the last Exp.
- softmax 1/sum via the single-op approximate reciprocal instead of the
  ~1.1us DVE reciprocal.
"""
import numpy as np

H = 64
N_AMB = 1023
N = 1024
HEADS = 4
RT = 4
APT = 3
SLOPE = 0.2
NCORES = 8
SHARD = 128

# bf16 blob column map (see _prep_inputs)
C_WL = 0        # top: W_l.T            [64, 256]
C_WR = 256      # top: W_r.T            [64, 256]
C_HT = 512      # top: hT (node 0 = h1 slot, zero), nodes j at col C_HT+j
C_WT = 0        # bottom: W_trans[t].T/3  [64, 256]
C_WSELF = 256   # bottom: W_self.T      [64, 64]
C_WML = 320     # bottom: W_merge[:, :64].T
C_WMR = 384     # bottom: W_merge[:, 64:].T
C_TA = 448      # bottom: type agents   [64, 12]
C_BTT = 460     # bottom: b_trans.T     [64, 4]
C_HID = 464     # bottom: hidden.T      [64, 1]
C_WD0B = 468    # bottom: Wd0[:, 64:].T
C_MLP = 532     # bottom: per-core mlp amb slice [64, 128]
C_WD0A = 660    # bottom: Wd0[:, :64].T
C_WD1 = 724     # bottom: Wd1.T         [64, 128]
C_WEXP = 1536   # full: block-diag w_attn  [128, 128]
C_WD2 = 1664    # full: Wd2.T           [128, 4]
CB = 1668

_compiled = None


def _build():
    import concourse.tile as tile
    from concourse import bacc, mybir

    f32 = mybir.dt.float32
    bf = mybir.dt.bfloat16
    AF = mybir.ActivationFunctionType
    ALU = mybir.AluOpType
    AX = mybir.AxisListType

    nc = bacc.Bacc("TRN2", target_bir_lowering=False, debug=False,
                   enable_asserts=False, num_devices=NCORES)

    bfb_d = nc.dram_tensor("bfb", [128, CB], bf, kind="ExternalInput").ap()
    f32b_d = nc.dram_tensor("f32b", [128, 5], f32, kind="ExternalInput").ap()
    outT_d = nc.dram_tensor("outT", [RT, SHARD], f32, kind="ExternalOutput").ap()

    with nc.allow_low_precision("bf16 kernel, tolerance 2e-2"), \
         tile.TileContext(nc) as tc:
        with tc.tile_pool(name="wp", bufs=1) as wp, \
             tc.tile_pool(name="sb", bufs=1) as sb, \
             tc.tile_pool(name="ps", bufs=1, space="PSUM") as ps:

            B = wp.tile([128, CB], bf, tag="bfb")
            F = wp.tile([128, 5], f32, tag="f32b")
            # ordered by when consumers need the data: prologue weights
            # first (the serial merge chain is the head of the critical
            # path), then biases, the full top half (W_l/W_r + hT), the
            # full-height tail (Wexp/fold/Wd2), and the MLP weights last.
            # Bottom cols 852:1536 are zeros and never transferred.
            nc.sync.dma_start(B[64:128, C_WSELF:C_WD0B], bfb_d[64:128, C_WSELF:C_WD0B])
            nc.sync.dma_start(B[64:128, 0:C_WSELF], bfb_d[64:128, 0:C_WSELF])
            nc.sync.dma_start(F[:], f32b_d[:])
            nc.sync.dma_start(B[0:64, 0:C_WEXP], bfb_d[0:64, 0:C_WEXP])
            nc.sync.dma_start(B[:, C_WEXP:CB], bfb_d[:, C_WEXP:CB])
            nc.sync.dma_start(B[64:128, C_WD0B:852], bfb_d[64:128, C_WD0B:852])

            bsc = F[64:128, 0:1]
            bmc = F[64:128, 1:2]
            bd0c = F[0:64, 2:3]
            bd1c = F[0:128, 3:4]
            bd2c = F[0:4, 4:5]

            # preload the Exp table off the critical path (Prelu/Identity/Exp
            # all live in the same table set)
            warm = wp.tile([1, 4], f32, tag="warm")
            nc.vector.memset(warm[:], 0.0)
            warm_act = wp.tile([1, 4], f32, tag="warmact")
            nc.scalar.activation(warm_act[0:1, 0:1], warm[0:1, 0:1], AF.Exp)

            # single-bank PSUM arena for all small matmul outputs (PSUM
            # allocation is bank-granular; separate tags would need a bank
            # each). Regions are disjoint columns of one [128, 512] bank.
            arena = ps.tile([128, 512], f32, tag="sp", bufs=1)
            y0_ps = arena[0:64, 0:SHARD]
            y1_ps = arena[0:128, 128:256]
            o_ps = arena[0:4, 256:384]
            h2p_ps = arena[0:64, 384:386]
            c0_ps = arena[0:64, 388:389]
            tmp_ps = arena[64:128, 392:396]
            C_ps = arena[64:128, 396:400]
            h1_ps = arena[64:128, 400:401]

            # ---- prologue ----
            # h1 = W_self @ hidden + b_self goes first: it heads the serial
            # merge chain, and the C-path below runs concurrently with it
            h1t = sb.tile([128, RT + 1], bf, tag="h1t")
            nc.tensor.matmul(h1_ps[:], B[64:128, C_WSELF:C_WSELF + H],
                             B[64:128, C_HID:C_HID + 1], start=True, stop=True,
                             tile_position=(64, 64))
            nc.scalar.activation(h1t[64:128, 0:1], h1_ps[:], AF.Identity,
                                 bias=bsc)

            # role-type routing (the per-iteration bias columns C_sb)
            tsum = sb.tile([128, RT], bf, tag="tsum")
            nc.vector.reduce_sum(
                tsum[64:128, :],
                B[64:128, C_TA:C_TA + RT * APT].rearrange("p (t a) -> p t a", a=APT),
                axis=AX.X)
            for t in range(RT):
                nc.tensor.matmul(tmp_ps[:, t:t + 1],
                                 B[64:128, C_WT + H * t:C_WT + H * (t + 1)],
                                 tsum[64:128, t:t + 1], start=True, stop=True,
                                 tile_position=(64, 64))
            tmpc = sb.tile([128, RT], bf, tag="tmpc")
            nc.vector.tensor_tensor(tmpc[64:128, :], tmp_ps[:],
                                    B[64:128, C_BTT:C_BTT + RT], op=ALU.add)
            nc.tensor.matmul(C_ps[:], B[64:128, C_WMR:C_WMR + H],
                             tmpc[64:128, :], start=True, stop=True,
                             tile_position=(64, 64))
            C_sb = sb.tile([128, RT], f32, tag="C")
            nc.vector.tensor_scalar_add(C_sb[64:128, :], C_ps[:], bmc)

            # 4x leaky-merge chain
            for t in range(RT):
                hp = arena[64:128, 402 + 2 * (t % 2):403 + 2 * (t % 2)]
                nc.tensor.matmul(hp, B[64:128, C_WML:C_WML + H],
                                 h1t[64:128, t:t + 1], start=True, stop=True,
                                 tile_position=(64, 64))
                if t < RT - 1:
                    nc.scalar.activation(h1t[64:128, t + 1:t + 2], hp,
                                         AF.Prelu, bias=C_sb[64:128, t:t + 1],
                                         alpha=SLOPE)
                else:
                    # final h1 -> node-0 column of hT (top half)
                    nc.scalar.activation(B[0:64, C_HT:C_HT + 1], hp,
                                         AF.Prelu, bias=C_sb[64:128, t:t + 1],
                                         alpha=SLOPE)

            # duplicated hT chunks: top = bottom = chunk, so one DVE pass
            # per unit can weight BOTH heads (pexp rows 0:64 and 64:128)
            # against the node features with all operands at base partition 0
            hdup = wp.tile([128, N], bf, tag="hdup")
            for c in range(2):
                cols = slice(C_HT + 512 * c, C_HT + 512 * (c + 1))
                nc.vector.tensor_copy(hdup[0:64, 512 * c:512 * (c + 1)], B[0:64, cols])
                nc.vector.tensor_copy(hdup[64:128, 512 * c:512 * (c + 1)], B[0:64, cols])

            # attention query columns g_r[0] per head-pair block
            gr0c = sb.tile([128, 2], f32, tag="gr0c")
            for b in range(2):
                gr0_ps = arena[0:128, 408 + 2 * b:409 + 2 * b]
                nc.tensor.matmul(gr0_ps, B[0:64, C_WR + 128 * b:C_WR + 128 * b + 128],
                                 B[0:64, C_HT:C_HT + 1], start=True, stop=True)
                nc.vector.tensor_copy(gr0c[:, b:b + 1], gr0_ps)

            # ---- first MLP matmul on this core's shard (h2-independent) ----
            nc.tensor.matmul(y0_ps, B[64:128, C_WD0A:C_WD0A + H],
                             B[64:128, C_MLP:C_MLP + SHARD], start=True, stop=True,
                             tile_position=(64, 0))

            # ---- GAT row 0: 2 head-pair blocks x 2 column chunks of 512.
            # Value aggregation uses linearity: sum_j a_j (W_r h_j) =
            # W_r (sum_j a_j h_j), so no big g_r matmuls are needed; the
            # weighted sums run on DVE straight against the bf16 hT columns
            # and W_r is applied once per head to a single 64-vector. ----
            ssum4 = sb.tile([128, 4], f32, tag="ssum4")
            vparts = sb.tile([128, 4], f32, tag="vparts")  # col = unit
            # pass 1: gl matmuls + fused leaky(gl + gr0) -> t_sb
            gl_list, t_list = [], []
            for b in range(2):
                for c in range(2):
                    cols = slice(C_HT + 512 * c, C_HT + 512 * (c + 1))
                    gl_ps = ps.tile([128, 512], f32, tag="ge", bufs=4)
                    nc.tensor.matmul(gl_ps[:],
                                     B[0:64, C_WL + 128 * b:C_WL + 128 * b + 128],
                                     B[0:64, cols], start=True, stop=True)
                    t_sb = sb.tile([128, 512], bf, tag="t", bufs=4)
                    nc.scalar.activation(t_sb[:], gl_ps[:], AF.Prelu,
                                         bias=gr0c[:, b:b + 1], alpha=SLOPE)
                    t_list.append(t_sb)
            # pass 2: attention logits -> exp -> per-head weighted node sums
            for b in range(2):
                for c in range(2):
                    u = 2 * b + c
                    cols = slice(C_HT + 512 * c, C_HT + 512 * (c + 1))
                    e_ps = ps.tile([128, 512], f32, tag="ge", bufs=4)
                    nc.tensor.matmul(e_ps[:], B[:, C_WEXP:C_WEXP + 128],
                                     t_list[u][:], start=True, stop=True)
                    pexp = sb.tile([128, 512], bf, tag="pexp", bufs=3)
                    nc.scalar.activation(pexp[:], e_ps[:], AF.Exp, bias=0.0,
                                         accum_out=ssum4[:, u:u + 1])
                    scr = sb.tile([128, 512], bf, tag="scr", bufs=3)
                    nc.vector.scalar_tensor_tensor(
                        out=scr[:], in0=pexp[:], scalar=1.0,
                        in1=hdup[:, 512 * c:512 * (c + 1)],
                        op0=ALU.mult, op1=ALU.mult,
                        accum_out=vparts[:, u:u + 1])

            # preload the Sigmoid table while the MLP matmuls run. Reading a
            # row of ssum4 makes this depend on ALL four Exp accumulators, so
            # the table switch is ordered strictly after the last Exp (the
            # same table also holds Prelu, so later Prelu ACTs don't reload).
            warm_sig = wp.tile([1, 4], f32, tag="warmsig")
            nc.scalar.activation(warm_sig[0:1, 0:4], ssum4[0:1, 0:4], AF.Sigmoid)

            # combine chunks, normalize (0.25 head-mean folded into the
            # reciprocal), apply W_r per head, accumulate h2 in PSUM
            ssum2 = sb.tile([128, 2], f32, tag="ssum2")
            v2 = sb.tile([128, 2], f32, tag="v2")
            for b in range(2):
                nc.gpsimd.tensor_tensor(ssum2[:, b:b + 1], ssum4[:, 2 * b:2 * b + 1],
                                        ssum4[:, 2 * b + 1:2 * b + 2], op=ALU.add)
                nc.gpsimd.tensor_tensor(v2[:, b:b + 1], vparts[:, 2 * b:2 * b + 1],
                                        vparts[:, 2 * b + 1:2 * b + 2], op=ALU.add)
            rs2 = sb.tile([128, 2], f32, tag="rs2")
            nc.vector.reciprocal_approx_fast(rs2[:], ssum2[:])
            rs2s = sb.tile([128, 2], f32, tag="rs2s")
            nc.vector.tensor_scalar_mul(rs2s[:], rs2[:], 1.0 / HEADS)
            vn2 = sb.tile([128, 2], bf, tag="vn2")
            nc.vector.tensor_tensor(vn2[:], v2[:], rs2s[:], op=ALU.mult)
            vnb = sb.tile([64, 2], bf, tag="vnb")
            nc.vector.tensor_copy(vnb[:], vn2[64:128, :])
            h2_ps = h2p_ps[:, 0:1]
            for h in range(RT):
                b, i = divmod(h, 2)
                rhs = vnb[:, b:b + 1] if i else vn2[0:64, b:b + 1]
                nc.tensor.matmul(h2_ps, B[0:64, C_WR + 64 * h:C_WR + 64 * h + 64],
                                 rhs, start=(h == 0), stop=(h == 3))
            h2b = sb.tile([128, 1], bf, tag="h2b")
            nc.vector.tensor_copy(h2b[64:128, :], h2_ps)

            # ---- final MLP (gated on h2) ----
            nc.tensor.matmul(c0_ps, B[64:128, C_WD0B:C_WD0B + H],
                             h2b[64:128, :], start=True, stop=True,
                             tile_position=(64, 0))
            c0col = sb.tile([64, 1], f32, tag="c0col")
            nc.vector.tensor_scalar_add(c0col[:], c0_ps, bd0c)
            y0b = sb.tile([128, SHARD], bf, tag="y0b")
            nc.scalar.activation(y0b[64:128, :], y0_ps, AF.Prelu, bias=c0col[:],
                                 alpha=SLOPE)
            nc.tensor.matmul(y1_ps, B[64:128, C_WD1:C_WD1 + SHARD],
                             y0b[64:128, :], start=True, stop=True,
                             tile_position=(64, 0))
            y1f = sb.tile([128, SHARD], bf, tag="y1f")
            nc.scalar.activation(y1f[:], y1_ps, AF.Prelu, bias=bd1c, alpha=SLOPE)
            nc.tensor.matmul(o_ps, B[:, C_WD2:C_WD2 + RT], y1f[:],
                             start=True, stop=True)
            o_sb = sb.tile([RT, SHARD], f32, tag="osb")
            nc.scalar.activation(o_sb[:], o_ps, AF.Sigmoid, bias=bd2c)
            nc.sync.dma_start(outT_d[:], o_sb[:])

    nc.compile()
    return nc


def _prep_inputs(inputs):
    import ml_dtypes
    bf16 = ml_dtypes.bfloat16
    f32 = np.float32

    hidden = np.asarray(inputs["hidden"], f32)
    ambiguous = np.asarray(inputs["ambiguous"], f32)
    type_agents = np.asarray(inputs["type_agents"], f32)
    W_self = np.asarray(inputs["W_self"], f32)
    b_self = np.asarray(inputs["b_self"], f32)
    W_merge = np.asarray(inputs["W_merge"], f32)
    b_merge = np.asarray(inputs["b_merge"], f32)
    W_trans = np.asarray(inputs["W_trans"], f32)
    b_trans = np.asarray(inputs["b_trans"], f32)
    W_l = np.asarray(inputs["W_l"], f32)
    W_r = np.asarray(inputs["W_r"], f32)
    w_attn = np.asarray(inputs["w_attn"], f32)
    Wd0 = np.asarray(inputs["Wd0"], f32)
    bd0 = np.asarray(inputs["bd0"], f32)
    Wd1 = np.asarray(inputs["Wd1"], f32)
    bd1 = np.asarray(inputs["bd1"], f32)
    Wd2 = np.asarray(inputs["Wd2"], f32)
    bd2 = np.asarray(inputs["bd2"], f32)

    base = np.zeros((128, CB), f32)
    top = base[0:64]
    bot = base[64:128]
    top[:, C_WL:C_WL + 256] = W_l.T
    top[:, C_WR:C_WR + 256] = W_r.T
    top[:, C_HT + 1:C_HT + N] = ambiguous.T
    bot[:, C_WT:C_WT + 256] = np.concatenate(
        [W_trans[t].T for t in range(RT)], axis=1) / APT
    bot[:, C_WSELF:C_WSELF + H] = W_self.T
    bot[:, C_WML:C_WML + H] = W_merge[:, :H].T
    bot[:, C_WMR:C_WMR + H] = W_merge[:, H:].T
    bot[:, C_WD0B:C_WD0B + H] = Wd0[:, H:].T
    bot[:, C_WD0A:C_WD0A + H] = Wd0[:, :H].T
    bot[:, C_WD1:C_WD1 + SHARD] = Wd1.T
    bot[:, C_TA:C_TA + RT * APT] = type_agents.reshape(RT * APT, H).T
    bot[:, C_BTT:C_BTT + RT] = b_trans.T
    bot[:, C_HID:C_HID + 1] = hidden.T
    wexp = np.zeros((128, 128), f32)
    for hh in range(2):
        wexp[hh * 64:(hh + 1) * 64, hh * 64:(hh + 1) * 64] = w_attn[:, None]
    base[:, C_WEXP:C_WEXP + 128] = wexp
    base[:, C_WD2:C_WD2 + RT] = Wd2.T

    f32b = np.zeros((128, 5), f32)
    f32b[64:128, 0] = b_self
    f32b[64:128, 1] = b_merge
    f32b[0:64, 2] = bd0
    f32b[0:128, 3] = bd1
    f32b[0:RT, 4] = bd2

    amb_pad = np.zeros((H, NCORES * SHARD), f32)
    amb_pad[:, :N_AMB] = ambiguous.T
    in_maps = []
    for cidx in range(NCORES):
        blob = base.copy()
        blob[64:128, C_MLP:C_MLP + SHARD] = \
            amb_pad[:, cidx * SHARD:(cidx + 1) * SHARD]
        in_maps.append({"bfb": blob.astype(bf16), "f32b": f32b})
    return in_maps


def kernel(**inputs) -> np.ndarray:
    global _compiled
    if _compiled is None:
        _compiled = _build()
    nc = _compiled
    from concourse import bass_utils

    in_maps = _prep_inputs(inputs)
    res = bass_utils.run_bass_kernel_spmd(nc, in_maps, core_ids=list(range(NCORES)))
    out = np.empty((N_AMB, RT), np.float32)
    for cidx in range(NCORES):
        lo = cidx * SHARD
        hi = min(lo + SHARD, N_AMB)
        out[lo:hi, :] = res.results[cidx]["outT"][:, :hi - lo].T
    return out


# revision 15
# speedup vs baseline: 2.1724x; 1.0068x over previous
"""Trainium2 Bass kernel for nn_MlroleNode_64716567216639 (GAT message passing).

Math note: the reference computes a dense NxN GATv2 attention but only row 0
of the output feeds the final MLP, so this kernel computes just that row:
e[j,h] = leaky(g_l[j] + g_r[0]) . w_attn over the 1024 source nodes, softmax,
weighted sum of g_r values, then the 3-layer type-define MLP over the 1023
ambiguous nodes (sharded 128 nodes per core; GAT row-0 replicated).

Optimizations vs the naive version:
- All inputs packed into ONE bf16 blob + one tiny fp32 blob -> 3 dma_starts
  instead of 22 (each dma_start costs ~600ns serially on the Sync engine).
- 64-row weights ride the unused bottom partitions (64:128) of the blob; the
  matmuls that consume them run in the lower PE quadrant via
  tile_position=(64, .).
- bf16 matmuls: single PE pass (fp32 runs LOW_HIGH = 4 passes).
- leaky(x + bias) fused into one scalar-engine ACT (Prelu, alpha=0.2) reading
  straight from PSUM. Prelu lives in the same ACT table as Exp -> no table
  switches; Sigmoid's table is preloaded via a dummy ACT after the last Exp.
- softmax 1/sum via the single-op approximate reciprocal instead of the
  ~1.1us DVE reciprocal.
"""
import numpy as np

H = 64
N_AMB = 1023
N = 1024
HEADS = 4
RT = 4
APT = 3
SLOPE = 0.2
NCORES = 8
SHARD = 128

# bf16 blob column map (see _prep_inputs)
C_WL = 0        # top: W_l.T            [64, 256]
C_WR = 256      # top: W_r.T            [64, 256]
C_HT = 512      # top: hT (node 0 = h1 slot, zero), nodes j at col C_HT+j
C_WT = 0        # bottom: W_trans[t].T/3  [64, 256]
C_WSELF = 256   # bottom: W_self.T      [64, 64]
C_WML = 320     # bottom: W_merge[:, :64].T
C_WMR = 384     # bottom: W_merge[:, 64:].T
C_TA = 448      # bottom: type agents   [64, 12]
C_BTT = 460     # bottom: b_trans.T     [64, 4]
C_HID = 464     # bottom: hidden.T      [64, 1]
C_BSC = 465     # bottom: b_self        [64, 1]
C_BMC = 466     # bottom: b_merge       [64, 1]
C_WD0B = 468    # bottom: Wd0[:, 64:].T
C_MLP = 532     # bottom: per-core mlp amb slice [64, 128]
C_WD0A = 660    # bottom: Wd0[:, :64].T
C_WD1 = 724     # bottom: Wd1.T         [64, 128]
C_WEXP = 1536   # full: block-diag w_attn  [128, 128]
C_WD2 = 1664    # full: Wd2.T           [128, 4]
C_BD1 = 1668    # full col: bd1
C_BD0 = 1669    # top rows 0:64: bd0
C_BD2 = 1670    # top rows 0:4: bd2
CB = 1671

_compiled = None


def _build():
    import concourse.tile as tile
    from concourse import bacc, mybir

    f32 = mybir.dt.float32
    bf = mybir.dt.bfloat16
    AF = mybir.ActivationFunctionType
    ALU = mybir.AluOpType
    AX = mybir.AxisListType

    nc = bacc.Bacc("TRN2", target_bir_lowering=False, debug=False,
                   enable_asserts=False, num_devices=NCORES)

    bfb_d = nc.dram_tensor("bfb", [128, CB], bf, kind="ExternalInput").ap()
    outT_d = nc.dram_tensor("outT", [RT, SHARD], f32, kind="ExternalOutput").ap()

    with nc.allow_low_precision("bf16 kernel, tolerance 2e-2"), \
         tile.TileContext(nc) as tc:
        with tc.tile_pool(name="wp", bufs=1) as wp, \
             tc.tile_pool(name="sb", bufs=1) as sb, \
             tc.tile_pool(name="ps", bufs=1, space="PSUM") as ps:

            B = wp.tile([128, CB], bf, tag="bfb")
            # ordered by when consumers need the data: prologue weights
            # first (the serial merge chain is the head of the critical
            # path), then biases, the full top half (W_l/W_r + hT), the
            # full-height tail (Wexp/fold/Wd2), and the MLP weights last.
            # Bottom cols 852:1536 are zeros and never transferred.
            nc.sync.dma_start(B[64:128, C_WSELF:C_WD0B], bfb_d[64:128, C_WSELF:C_WD0B])
            nc.sync.dma_start(B[64:128, 0:C_WSELF], bfb_d[64:128, 0:C_WSELF])
            nc.sync.dma_start(B[0:64, 0:C_WEXP], bfb_d[0:64, 0:C_WEXP])
            nc.sync.dma_start(B[:, C_WEXP:CB], bfb_d[:, C_WEXP:CB])
            nc.sync.dma_start(B[64:128, C_WD0B:852], bfb_d[64:128, C_WD0B:852])

            # biases travel as bf16 in the blob; DVE ops need fp32 scalar
            # operands, so widen them into small fp32 tiles right after the
            # carrying DMA lands (all off the critical path)
            biasP = sb.tile([128, 2], f32, tag="biasP")
            nc.vector.tensor_copy(biasP[64:128, :], B[64:128, C_BSC:C_BSC + 2])
            bsc = biasP[64:128, 0:1]
            bmc = biasP[64:128, 1:2]
            biasM = sb.tile([128, 3], f32, tag="biasM")
            nc.vector.tensor_copy(biasM[:, 0:1], B[0:128, C_BD1:C_BD1 + 1])
            nc.vector.tensor_copy(biasM[0:64, 1:3], B[0:64, C_BD0:C_BD0 + 2])
            bd1c = biasM[0:128, 0:1]
            bd0c = biasM[0:64, 1:2]
            bd2c = biasM[0:4, 2:3]

            # preload the Exp table off the critical path (Prelu/Identity/Exp
            # all live in the same table set)
            warm = wp.tile([1, 4], f32, tag="warm")
            nc.vector.memset(warm[:], 0.0)
            warm_act = wp.tile([1, 4], f32, tag="warmact")
            nc.scalar.activation(warm_act[0:1, 0:1], warm[0:1, 0:1], AF.Exp)

            # PSUM arenas for small matmul outputs (bank-granular alloc).
            # Two separate banks: the tile-level dependency tracking adds
            # false write-after-read ordering between unrelated regions of
            # one tile, which serialized the prologue when shared.
            arenaP = ps.tile([128, 512], f32, tag="spP", bufs=1)
            tmp_ps = arenaP[64:128, 0:4]
            C_ps = arenaP[64:128, 8:12]
            h1_ps = arenaP[64:128, 16:17]
            arenaM = ps.tile([128, 512], f32, tag="spM", bufs=1)
            y0_ps = arenaM[0:64, 0:SHARD]
            y1_ps = arenaM[0:128, 128:256]
            o_ps = arenaM[0:4, 256:384]
            h2p_ps = arenaM[0:64, 384:386]
            c0_ps = arenaM[0:64, 388:389]

            # ---- prologue ----
            # h1 = W_self @ hidden + b_self goes first: it heads the serial
            # merge chain, and the C-path below runs concurrently with it
            h1t = sb.tile([128, RT + 1], bf, tag="h1t")
            nc.tensor.matmul(h1_ps[:], B[64:128, C_WSELF:C_WSELF + H],
                             B[64:128, C_HID:C_HID + 1], start=True, stop=True,
                             tile_position=(64, 64))
            nc.scalar.activation(h1t[64:128, 0:1], h1_ps[:], AF.Identity,
                                 bias=bsc)

            # role-type routing (the per-iteration bias columns C_sb)
            tsum = sb.tile([128, RT], bf, tag="tsum")
            nc.vector.reduce_sum(
                tsum[64:128, :],
                B[64:128, C_TA:C_TA + RT * APT].rearrange("p (t a) -> p t a", a=APT),
                axis=AX.X)
            for t in range(RT):
                nc.tensor.matmul(tmp_ps[:, t:t + 1],
                                 B[64:128, C_WT + H * t:C_WT + H * (t + 1)],
                                 tsum[64:128, t:t + 1], start=True, stop=True,
                                 tile_position=(64, 64))
            tmpc = sb.tile([128, RT], bf, tag="tmpc")
            nc.vector.tensor_tensor(tmpc[64:128, :], tmp_ps[:],
                                    B[64:128, C_BTT:C_BTT + RT], op=ALU.add)
            nc.tensor.matmul(C_ps[:], B[64:128, C_WMR:C_WMR + H],
                             tmpc[64:128, :], start=True, stop=True,
                             tile_position=(64, 64))
            C_sb = sb.tile([128, RT], f32, tag="C")
            nc.vector.tensor_scalar_add(C_sb[64:128, :], C_ps[:], bmc)

            # 4x leaky-merge chain
            for t in range(RT):
                hp = arenaP[64:128, 20 + 2 * (t % 2):21 + 2 * (t % 2)]
                nc.tensor.matmul(hp, B[64:128, C_WML:C_WML + H],
                                 h1t[64:128, t:t + 1], start=True, stop=True,
                                 tile_position=(64, 64))
                if t < RT - 1:
                    nc.scalar.activation(h1t[64:128, t + 1:t + 2], hp,
                                         AF.Prelu, bias=C_sb[64:128, t:t + 1],
                                         alpha=SLOPE)
                else:
                    # final h1 -> node-0 column of hT (top half)
                    nc.scalar.activation(B[0:64, C_HT:C_HT + 1], hp,
                                         AF.Prelu, bias=C_sb[64:128, t:t + 1],
                                         alpha=SLOPE)

            # duplicated hT chunks: top = bottom = chunk, so one DVE pass
            # per unit can weight BOTH heads (pexp rows 0:64 and 64:128)
            # against the node features with all operands at base partition 0
            hdup = wp.tile([128, N], bf, tag="hdup")
            for c in range(2):
                cols = slice(C_HT + 512 * c, C_HT + 512 * (c + 1))
                nc.vector.tensor_copy(hdup[0:64, 512 * c:512 * (c + 1)], B[0:64, cols])
                nc.vector.tensor_copy(hdup[64:128, 512 * c:512 * (c + 1)], B[0:64, cols])

            # attention query columns g_r[0] per head-pair block
            gr0c = sb.tile([128, 2], f32, tag="gr0c")
            for b in range(2):
                gr0_ps = ps.tile([128, 1], f32, tag="gr0", bufs=2)
                nc.tensor.matmul(gr0_ps[:], B[0:64, C_WR + 128 * b:C_WR + 128 * b + 128],
                                 B[0:64, C_HT:C_HT + 1], start=True, stop=True)
                nc.vector.tensor_copy(gr0c[:, b:b + 1], gr0_ps[:])

            # ---- first MLP matmul on this core's shard (h2-independent) ----
            nc.tensor.matmul(y0_ps, B[64:128, C_WD0A:C_WD0A + H],
                             B[64:128, C_MLP:C_MLP + SHARD], start=True, stop=True,
                             tile_position=(64, 0))

            # ---- GAT row 0: 2 head-pair blocks x 2 column chunks of 512.
            # Value aggregation uses linearity: sum_j a_j (W_r h_j) =
            # W_r (sum_j a_j h_j), so no big g_r matmuls are needed; the
            # weighted sums run on DVE straight against the bf16 hT columns
            # and W_r is applied once per head to a single 64-vector. ----
            ssum4 = sb.tile([128, 4], f32, tag="ssum4")
            vparts = sb.tile([128, 4], f32, tag="vparts")  # col = unit
            # pass 1: gl matmuls + fused leaky(gl + gr0) -> t_sb
            gl_list, t_list = [], []
            for b in range(2):
                for c in range(2):
                    cols = slice(C_HT + 512 * c, C_HT + 512 * (c + 1))
                    gl_ps = ps.tile([128, 512], f32, tag="ge", bufs=4)
                    nc.tensor.matmul(gl_ps[:],
                                     B[0:64, C_WL + 128 * b:C_WL + 128 * b + 128],
                                     B[0:64, cols], start=True, stop=True)
                    t_sb = sb.tile([128, 512], bf, tag="t", bufs=4)
                    nc.scalar.activation(t_sb[:], gl_ps[:], AF.Prelu,
                                         bias=gr0c[:, b:b + 1], alpha=SLOPE)
                    t_list.append(t_sb)
            # pass 2: attention logits -> exp -> per-head weighted node sums
            for b in range(2):
                for c in range(2):
                    u = 2 * b + c
                    cols = slice(C_HT + 512 * c, C_HT + 512 * (c + 1))
                    e_ps = ps.tile([128, 512], f32, tag="ge", bufs=4)
                    nc.tensor.matmul(e_ps[:], B[:, C_WEXP:C_WEXP + 128],
                                     t_list[u][:], start=True, stop=True)
                    pexp = sb.tile([128, 512], bf, tag="pexp", bufs=3)
                    nc.scalar.activation(pexp[:], e_ps[:], AF.Exp, bias=0.0,
                                         accum_out=ssum4[:, u:u + 1])
                    scr = sb.tile([128, 512], bf, tag="scr", bufs=3)
                    nc.vector.scalar_tensor_tensor(
                        out=scr[:], in0=pexp[:], scalar=1.0,
                        in1=hdup[:, 512 * c:512 * (c + 1)],
                        op0=ALU.mult, op1=ALU.mult,
                        accum_out=vparts[:, u:u + 1])

            # preload the Sigmoid table while the MLP matmuls run. Reading a
            # row of ssum4 makes this depend on ALL four Exp accumulators, so
            # the table switch is ordered strictly after the last Exp (the
            # same table also holds Prelu, so later Prelu ACTs don't reload).
            warm_sig = wp.tile([1, 4], f32, tag="warmsig")
            nc.scalar.activation(warm_sig[0:1, 0:4], ssum4[0:1, 0:4], AF.Sigmoid)

            # combine chunks, normalize (0.25 head-mean folded into the
            # reciprocal), apply W_r per head, accumulate h2 in PSUM
            ssum2 = sb.tile([128, 2], f32, tag="ssum2")
            v2 = sb.tile([128, 2], f32, tag="v2")
            for b in range(2):
                nc.vector.tensor_tensor(ssum2[:, b:b + 1], ssum4[:, 2 * b:2 * b + 1],
                                        ssum4[:, 2 * b + 1:2 * b + 2], op=ALU.add)
                nc.vector.tensor_tensor(v2[:, b:b + 1], vparts[:, 2 * b:2 * b + 1],
                                        vparts[:, 2 * b + 1:2 * b + 2], op=ALU.add)
            rs2 = sb.tile([128, 2], f32, tag="rs2")
            nc.vector.reciprocal_approx_fast(rs2[:], ssum2[:])
            rs2s = sb.tile([128, 2], f32, tag="rs2s")
            nc.vector.tensor_scalar_mul(rs2s[:], rs2[:], 1.0 / HEADS)
            vn2 = sb.tile([128, 2], bf, tag="vn2")
            nc.vector.tensor_tensor(vn2[:], v2[:], rs2s[:], op=ALU.mult)
            vnb = sb.tile([64, 2], bf, tag="vnb")
            nc.vector.tensor_copy(vnb[:], vn2[64:128, :])
            h2_ps = h2p_ps[:, 0:1]
            for h in range(RT):
                b, i = divmod(h, 2)
                rhs = vnb[:, b:b + 1] if i else vn2[0:64, b:b + 1]
                nc.tensor.matmul(h2_ps, B[0:64, C_WR + 64 * h:C_WR + 64 * h + 64],
                                 rhs, start=(h == 0), stop=(h == 3))
            h2b = sb.tile([128, 1], bf, tag="h2b")
            nc.vector.tensor_copy(h2b[64:128, :], h2_ps)

            # ---- final MLP (gated on h2) ----
            nc.tensor.matmul(c0_ps, B[64:128, C_WD0B:C_WD0B + H],
                             h2b[64:128, :], start=True, stop=True,
                             tile_position=(64, 0))
            c0col = sb.tile([64, 1], f32, tag="c0col")
            nc.vector.tensor_scalar_add(c0col[:], c0_ps, bd0c)
            y0b = sb.tile([128, SHARD], bf, tag="y0b")
            nc.scalar.activation(y0b[64:128, :], y0_ps, AF.Prelu, bias=c0col[:],
                                 alpha=SLOPE)
            nc.tensor.matmul(y1_ps, B[64:128, C_WD1:C_WD1 + SHARD],
                             y0b[64:128, :], start=True, stop=True,
                             tile_position=(64, 0))
            y1f = sb.tile([128, SHARD], bf, tag="y1f")
            nc.scalar.activation(y1f[:], y1_ps, AF.Prelu, bias=bd1c, alpha=SLOPE)
            nc.tensor.matmul(o_ps, B[:, C_WD2:C_WD2 + RT], y1f[:],
                             start=True, stop=True)
            o_sb = sb.tile([RT, SHARD], f32, tag="osb")
            nc.scalar.activation(o_sb[:], o_ps, AF.Sigmoid, bias=bd2c)
            nc.sync.dma_start(outT_d[:], o_sb[:])

    nc.compile()
    return nc


def _prep_inputs(inputs):
    import ml_dtypes
    bf16 = ml_dtypes.bfloat16
    f32 = np.float32

    hidden = np.asarray(inputs["hidden"], f32)
    ambiguous = np.asarray(inputs["ambiguous"], f32)
    type_agents = np.asarray(inputs["type_agents"], f32)
    W_self = np.asarray(inputs["W_self"], f32)
    b_self = np.asarray(inputs["b_self"], f32)
    W_merge = np.asarray(inputs["W_merge"], f32)
    b_merge = np.asarray(inputs["b_merge"], f32)
    W_trans = np.asarray(inputs["W_trans"], f32)
    b_trans = np.asarray(inputs["b_trans"], f32)
    W_l = np.asarray(inputs["W_l"], f32)
    W_r = np.asarray(inputs["W_r"], f32)
    w_attn = np.asarray(inputs["w_attn"], f32)
    Wd0 = np.asarray(inputs["Wd0"], f32)
    bd0 = np.asarray(inputs["bd0"], f32)
    Wd1 = np.asarray(inputs["Wd1"], f32)
    bd1 = np.asarray(inputs["bd1"], f32)
    Wd2 = np.asarray(inputs["Wd2"], f32)
    bd2 = np.asarray(inputs["bd2"], f32)

    base = np.zeros((128, CB), f32)
    top = base[0:64]
    bot = base[64:128]
    top[:, C_WL:C_WL + 256] = W_l.T
    top[:, C_WR:C_WR + 256] = W_r.T
    top[:, C_HT + 1:C_HT + N] = ambiguous.T
    bot[:, C_WT:C_WT + 256] = np.concatenate(
        [W_trans[t].T for t in range(RT)], axis=1) / APT
    bot[:, C_WSELF:C_WSELF + H] = W_self.T
    bot[:, C_WML:C_WML + H] = W_merge[:, :H].T
    bot[:, C_WMR:C_WMR + H] = W_merge[:, H:].T
    bot[:, C_WD0B:C_WD0B + H] = Wd0[:, H:].T
    bot[:, C_WD0A:C_WD0A + H] = Wd0[:, :H].T
    bot[:, C_WD1:C_WD1 + SHARD] = Wd1.T
    bot[:, C_TA:C_TA + RT * APT] = type_agents.reshape(RT * APT, H).T
    bot[:, C_BTT:C_BTT + RT] = b_trans.T
    bot[:, C_HID:C_HID + 1] = hidden.T
    bot[:, C_BSC] = b_self
    bot[:, C_BMC] = b_merge
    wexp = np.zeros((128, 128), f32)
    for hh in range(2):
        wexp[hh * 64:(hh + 1) * 64, hh * 64:(hh + 1) * 64] = w_attn[:, None]
    base[:, C_WEXP:C_WEXP + 128] = wexp
    base[:, C_WD2:C_WD2 + RT] = Wd2.T
    base[:, C_BD1] = bd1
    top[:, C_BD0] = bd0
    base[0:RT, C_BD2] = bd2

    amb_pad = np.zeros((H, NCORES * SHARD), f32)
    amb_pad[:, :N_AMB] = ambiguous.T
    in_maps = []
    for cidx in range(NCORES):
        blob = base.copy()
        blob[64:128, C_MLP:C_MLP + SHARD] = \
            amb_pad[:, cidx * SHARD:(cidx + 1) * SHARD]
        in_maps.append({"bfb": blob.astype(bf16)})
    return in_maps


def kernel(**inputs) -> np.ndarray:
    global _compiled
    if _compiled is None:
        _compiled = _build()
    nc = _compiled
    from concourse import bass_utils

    in_maps = _prep_inputs(inputs)
    res = bass_utils.run_bass_kernel_spmd(nc, in_maps, core_ids=list(range(NCORES)))
    out = np.empty((N_AMB, RT), np.float32)
    for cidx in range(NCORES):
        lo = cidx * SHARD
        hi = min(lo + SHARD, N_AMB)
        out[lo:hi, :] = res.results[cidx]["outT"][:, :hi - lo].T
    return out


# revision 16
# speedup vs baseline: 2.1936x; 1.0098x over previous
"""Trainium2 Bass kernel for nn_MlroleNode_64716567216639 (GAT message passing).

Math note: the reference computes a dense NxN GATv2 attention but only row 0
of the output feeds the final MLP, so this kernel computes just that row:
e[j,h] = leaky(g_l[j] + g_r[0]) . w_attn over the 1024 source nodes, softmax,
weighted sum of g_r values, then the 3-layer type-define MLP over the 1023
ambiguous nodes (sharded 128 nodes per core; GAT row-0 replicated).

Optimizations vs the naive version:
- All inputs packed into ONE bf16 blob + one tiny fp32 blob -> 3 dma_starts
  instead of 22 (each dma_start costs ~600ns serially on the Sync engine).
- 64-row weights ride the unused bottom partitions (64:128) of the blob; the
  matmuls that consume them run in the lower PE quadrant via
  tile_position=(64, .).
- bf16 matmuls: single PE pass (fp32 runs LOW_HIGH = 4 passes).
- leaky(x + bias) fused into one scalar-engine ACT (Prelu, alpha=0.2) reading
  straight from PSUM. Prelu lives in the same ACT table as Exp -> no table
  switches; Sigmoid's table is preloaded via a dummy ACT after the last Exp.
- softmax 1/sum via the single-op approximate reciprocal instead of the
  ~1.1us DVE reciprocal.
"""
import numpy as np

H = 64
N_AMB = 1023
N = 1024
HEADS = 4
RT = 4
APT = 3
SLOPE = 0.2
NCORES = 8
SHARD = 128

# bf16 blob column map (see _prep_inputs)
C_WL = 0        # top: W_l.T            [64, 256]
C_WR = 256      # top: W_r.T            [64, 256]
C_HT = 512      # top: hT (node 0 = h1 slot, zero), nodes j at col C_HT+j
C_WT = 0        # bottom: W_trans[t].T/3  [64, 256]
C_WSELF = 256   # bottom: W_self.T      [64, 64]
C_WML = 320     # bottom: W_merge[:, :64].T
C_WMR = 384     # bottom: W_merge[:, 64:].T
C_TA = 448      # bottom: type agents   [64, 12]
C_BTT = 460     # bottom: b_trans.T     [64, 4]
C_HID = 464     # bottom: hidden.T      [64, 1]
C_BSC = 465     # bottom: b_self        [64, 1]
C_BMC = 466     # bottom: b_merge       [64, 1]
C_WD0B = 468    # bottom: Wd0[:, 64:].T
C_MLP = 532     # bottom: per-core mlp amb slice [64, 128]
C_WD0A = 660    # bottom: Wd0[:, :64].T
C_WD1 = 724     # bottom: Wd1.T         [64, 128]
C_WEXP = 1536   # full: block-diag w_attn  [128, 128]
C_WD2 = 1664    # full: Wd2.T           [128, 4]
C_BD1 = 1668    # full col: bd1
C_BD0 = 1669    # top rows 0:64: bd0
C_BD2 = 1670    # top rows 0:4: bd2
CB = 1671

_compiled = None


def _build():
    import concourse.tile as tile
    from concourse import bacc, mybir

    f32 = mybir.dt.float32
    bf = mybir.dt.bfloat16
    AF = mybir.ActivationFunctionType
    ALU = mybir.AluOpType
    AX = mybir.AxisListType

    nc = bacc.Bacc("TRN2", target_bir_lowering=False, debug=False,
                   enable_asserts=False, num_devices=NCORES)

    bfb_d = nc.dram_tensor("bfb", [128, CB], bf, kind="ExternalInput").ap()
    outT_d = nc.dram_tensor("outT", [RT, SHARD], f32, kind="ExternalOutput").ap()

    with nc.allow_low_precision("bf16 kernel, tolerance 2e-2"), \
         tile.TileContext(nc) as tc:
        with tc.tile_pool(name="wp", bufs=1) as wp, \
             tc.tile_pool(name="sb", bufs=1) as sb, \
             tc.tile_pool(name="ps", bufs=1, space="PSUM") as ps:

            B = wp.tile([128, CB], bf, tag="bfb")
            # ordered by when consumers need the data: prologue weights
            # first (the serial merge chain is the head of the critical
            # path), then biases, the full top half (W_l/W_r + hT), the
            # full-height tail (Wexp/fold/Wd2), and the MLP weights last.
            # Bottom cols 852:1536 are zeros and never transferred.
            nc.sync.dma_start(B[64:128, C_WSELF:C_WD0B], bfb_d[64:128, C_WSELF:C_WD0B])
            nc.sync.dma_start(B[64:128, 0:C_WSELF], bfb_d[64:128, 0:C_WSELF])
            nc.sync.dma_start(B[0:64, 0:C_WEXP], bfb_d[0:64, 0:C_WEXP])
            nc.sync.dma_start(B[:, C_WEXP:CB], bfb_d[:, C_WEXP:CB])
            nc.sync.dma_start(B[64:128, C_WD0B:852], bfb_d[64:128, C_WD0B:852])

            # biases travel as bf16 in the blob; DVE ops need fp32 scalar
            # operands, so widen them into small fp32 tiles right after the
            # carrying DMA lands (all off the critical path)
            biasP = sb.tile([128, 2], f32, tag="biasP")
            nc.vector.tensor_copy(biasP[64:128, :], B[64:128, C_BSC:C_BSC + 2])
            bsc = biasP[64:128, 0:1]
            bmc = biasP[64:128, 1:2]
            biasM = sb.tile([128, 3], f32, tag="biasM")
            nc.vector.tensor_copy(biasM[:, 0:1], B[0:128, C_BD1:C_BD1 + 1])
            nc.vector.tensor_copy(biasM[0:64, 1:3], B[0:64, C_BD0:C_BD0 + 2])
            bd1c = biasM[0:128, 0:1]
            bd0c = biasM[0:64, 1:2]
            bd2c = biasM[0:4, 2:3]

            # preload the Exp table off the critical path (Prelu/Identity/Exp
            # all live in the same table set)
            warm = wp.tile([1, 4], f32, tag="warm")
            nc.vector.memset(warm[:], 0.0)
            warm_act = wp.tile([1, 4], f32, tag="warmact")
            nc.scalar.activation(warm_act[0:1, 0:1], warm[0:1, 0:1], AF.Exp)

            # PSUM arenas for small matmul outputs (bank-granular alloc).
            # Two separate banks: the tile-level dependency tracking adds
            # false write-after-read ordering between unrelated regions of
            # one tile, which serialized the prologue when shared.
            arenaM = ps.tile([128, 512], f32, tag="spM", bufs=1)
            tmp_ps = arenaM[64:128, 392:396]
            C_ps = arenaM[64:128, 400:404]
            y0_ps = arenaM[0:64, 0:SHARD]
            y1_ps = arenaM[0:128, 128:256]
            o_ps = arenaM[0:4, 256:384]
            h2p_ps = arenaM[0:64, 384:386]
            c0_ps = arenaM[0:64, 388:389]
            h1_ps = ps.tile([128, 1], f32, tag="hp", bufs=2)

            # ---- prologue ----
            # h1 = W_self @ hidden + b_self goes first: it heads the serial
            # merge chain, and the C-path below runs concurrently with it
            h1t = sb.tile([128, RT + 1], bf, tag="h1t")
            nc.tensor.matmul(h1_ps[64:128, :], B[64:128, C_WSELF:C_WSELF + H],
                             B[64:128, C_HID:C_HID + 1], start=True, stop=True,
                             tile_position=(64, 64))
            nc.scalar.activation(h1t[64:128, 0:1], h1_ps[64:128, :], AF.Identity,
                                 bias=bsc)

            # role-type routing (the per-iteration bias columns C_sb)
            tsum = sb.tile([128, RT], bf, tag="tsum")
            nc.vector.reduce_sum(
                tsum[64:128, :],
                B[64:128, C_TA:C_TA + RT * APT].rearrange("p (t a) -> p t a", a=APT),
                axis=AX.X)
            for t in range(RT):
                nc.tensor.matmul(tmp_ps[:, t:t + 1],
                                 B[64:128, C_WT + H * t:C_WT + H * (t + 1)],
                                 tsum[64:128, t:t + 1], start=True, stop=True,
                                 tile_position=(64, 64))
            tmpc = sb.tile([128, RT], bf, tag="tmpc")
            nc.vector.tensor_tensor(tmpc[64:128, :], tmp_ps[:],
                                    B[64:128, C_BTT:C_BTT + RT], op=ALU.add)
            nc.tensor.matmul(C_ps[:], B[64:128, C_WMR:C_WMR + H],
                             tmpc[64:128, :], start=True, stop=True,
                             tile_position=(64, 64))
            C_sb = sb.tile([128, RT], f32, tag="C")
            nc.vector.tensor_scalar_add(C_sb[64:128, :], C_ps[:], bmc)

            # 4x leaky-merge chain
            for t in range(RT):
                hp = ps.tile([128, 1], f32, tag="hp", bufs=2)
                nc.tensor.matmul(hp[64:128, :], B[64:128, C_WML:C_WML + H],
                                 h1t[64:128, t:t + 1], start=True, stop=True,
                                 tile_position=(64, 64))
                if t < RT - 1:
                    nc.scalar.activation(h1t[64:128, t + 1:t + 2], hp[64:128, :],
                                         AF.Prelu, bias=C_sb[64:128, t:t + 1],
                                         alpha=SLOPE)
                else:
                    # final h1 -> node-0 column of hT (top half)
                    nc.scalar.activation(B[0:64, C_HT:C_HT + 1], hp[64:128, :],
                                         AF.Prelu, bias=C_sb[64:128, t:t + 1],
                                         alpha=SLOPE)

            # attention query columns g_r[0] per head-pair block
            gr0c = sb.tile([128, 2], f32, tag="gr0c")
            for b in range(2):
                gr0_ps = ps.tile([128, 1], f32, tag="gr0", bufs=2)
                nc.tensor.matmul(gr0_ps[:], B[0:64, C_WR + 128 * b:C_WR + 128 * b + 128],
                                 B[0:64, C_HT:C_HT + 1], start=True, stop=True)
                nc.vector.tensor_copy(gr0c[:, b:b + 1], gr0_ps[:])

            # duplicated hT chunks: top = bottom = chunk, so one DVE pass
            # per unit can weight BOTH heads (pexp rows 0:64 and 64:128)
            # against the node features with all operands at base partition 0
            hdup = wp.tile([128, N], bf, tag="hdup")
            for c in range(2):
                cols = slice(C_HT + 512 * c, C_HT + 512 * (c + 1))
                nc.vector.tensor_copy(hdup[0:64, 512 * c:512 * (c + 1)], B[0:64, cols])
                nc.vector.tensor_copy(hdup[64:128, 512 * c:512 * (c + 1)], B[0:64, cols])


            # ---- first MLP matmul on this core's shard (h2-independent) ----
            nc.tensor.matmul(y0_ps, B[64:128, C_WD0A:C_WD0A + H],
                             B[64:128, C_MLP:C_MLP + SHARD], start=True, stop=True,
                             tile_position=(64, 0))

            # ---- GAT row 0: 2 head-pair blocks x 2 column chunks of 512.
            # Value aggregation uses linearity: sum_j a_j (W_r h_j) =
            # W_r (sum_j a_j h_j), so no big g_r matmuls are needed; the
            # weighted sums run on DVE straight against the bf16 hT columns
            # and W_r is applied once per head to a single 64-vector. ----
            ssum4 = sb.tile([128, 4], f32, tag="ssum4")
            vparts = sb.tile([128, 4], f32, tag="vparts")  # col = unit
            # pass 1: gl matmuls + fused leaky(gl + gr0) -> t_sb
            gl_list, t_list = [], []
            for b in range(2):
                for c in range(2):
                    cols = slice(C_HT + 512 * c, C_HT + 512 * (c + 1))
                    gl_ps = ps.tile([128, 512], f32, tag="ge", bufs=3)
                    nc.tensor.matmul(gl_ps[:],
                                     B[0:64, C_WL + 128 * b:C_WL + 128 * b + 128],
                                     B[0:64, cols], start=True, stop=True)
                    t_sb = sb.tile([128, 512], bf, tag="t", bufs=4)
                    nc.scalar.activation(t_sb[:], gl_ps[:], AF.Prelu,
                                         bias=gr0c[:, b:b + 1], alpha=SLOPE)
                    t_list.append(t_sb)
            # pass 2: attention logits -> exp -> per-head weighted node sums
            for b in range(2):
                for c in range(2):
                    u = 2 * b + c
                    cols = slice(C_HT + 512 * c, C_HT + 512 * (c + 1))
                    e_ps = ps.tile([128, 512], f32, tag="ge", bufs=3)
                    nc.tensor.matmul(e_ps[:], B[:, C_WEXP:C_WEXP + 128],
                                     t_list[u][:], start=True, stop=True)
                    pexp = sb.tile([128, 512], bf, tag="pexp", bufs=3)
                    nc.scalar.activation(pexp[:], e_ps[:], AF.Exp, bias=0.0,
                                         accum_out=ssum4[:, u:u + 1])
                    scr = sb.tile([128, 512], bf, tag="scr", bufs=3)
                    nc.vector.scalar_tensor_tensor(
                        out=scr[:], in0=pexp[:], scalar=1.0,
                        in1=hdup[:, 512 * c:512 * (c + 1)],
                        op0=ALU.mult, op1=ALU.mult,
                        accum_out=vparts[:, u:u + 1])

            # preload the Sigmoid table while the MLP matmuls run. Reading a
            # row of ssum4 makes this depend on ALL four Exp accumulators, so
            # the table switch is ordered strictly after the last Exp (the
            # same table also holds Prelu, so later Prelu ACTs don't reload).
            warm_sig = wp.tile([1, 4], f32, tag="warmsig")
            nc.scalar.activation(warm_sig[0:1, 0:4], ssum4[0:1, 0:4], AF.Sigmoid)

            # combine chunks, normalize (0.25 head-mean folded into the
            # reciprocal), apply W_r per head, accumulate h2 in PSUM
            ssum2 = sb.tile([128, 2], f32, tag="ssum2")
            v2 = sb.tile([128, 2], f32, tag="v2")
            sview = ssum4[:].rearrange("p (b c) -> p b c", c=2)
            vview = vparts[:].rearrange("p (b c) -> p b c", c=2)
            nc.vector.tensor_tensor(ssum2[:], sview[:, :, 0], sview[:, :, 1],
                                    op=ALU.add)
            nc.vector.tensor_tensor(v2[:], vview[:, :, 0], vview[:, :, 1],
                                    op=ALU.add)
            rs2 = sb.tile([128, 2], f32, tag="rs2")
            nc.vector.reciprocal_approx_fast(rs2[:], ssum2[:])
            vn2 = sb.tile([128, 2], bf, tag="vn2")
            # vn = v * (1/sum) * 0.25 (head mean) in one fused DVE op
            nc.vector.scalar_tensor_tensor(out=vn2[:], in0=v2[:],
                                           scalar=1.0 / HEADS, in1=rs2[:],
                                           op0=ALU.mult, op1=ALU.mult)
            vnb = sb.tile([64, 2], bf, tag="vnb")
            nc.vector.tensor_copy(vnb[:], vn2[64:128, :])
            h2_ps = h2p_ps[:, 0:1]
            for h in range(RT):
                b, i = divmod(h, 2)
                rhs = vnb[:, b:b + 1] if i else vn2[0:64, b:b + 1]
                nc.tensor.matmul(h2_ps, B[0:64, C_WR + 64 * h:C_WR + 64 * h + 64],
                                 rhs, start=(h == 0), stop=(h == 3))
            h2b = sb.tile([128, 1], bf, tag="h2b")
            nc.vector.tensor_copy(h2b[64:128, :], h2_ps)

            # ---- final MLP (gated on h2) ----
            nc.tensor.matmul(c0_ps, B[64:128, C_WD0B:C_WD0B + H],
                             h2b[64:128, :], start=True, stop=True,
                             tile_position=(64, 0))
            c0col = sb.tile([64, 1], f32, tag="c0col")
            nc.vector.tensor_scalar_add(c0col[:], c0_ps, bd0c)
            y0b = sb.tile([128, SHARD], bf, tag="y0b")
            nc.scalar.activation(y0b[64:128, :], y0_ps, AF.Prelu, bias=c0col[:],
                                 alpha=SLOPE)
            nc.tensor.matmul(y1_ps, B[64:128, C_WD1:C_WD1 + SHARD],
                             y0b[64:128, :], start=True, stop=True,
                             tile_position=(64, 0))
            y1f = sb.tile([128, SHARD], bf, tag="y1f")
            nc.scalar.activation(y1f[:], y1_ps, AF.Prelu, bias=bd1c, alpha=SLOPE)
            nc.tensor.matmul(o_ps, B[:, C_WD2:C_WD2 + RT], y1f[:],
                             start=True, stop=True)
            o_sb = sb.tile([RT, SHARD], f32, tag="osb")
            nc.scalar.activation(o_sb[:], o_ps, AF.Sigmoid, bias=bd2c)
            nc.sync.dma_start(outT_d[:], o_sb[:])

    nc.compile()
    return nc


def _prep_inputs(inputs):
    import ml_dtypes
    bf16 = ml_dtypes.bfloat16
    f32 = np.float32

    hidden = np.asarray(inputs["hidden"], f32)
    ambiguous = np.asarray(inputs["ambiguous"], f32)
    type_agents = np.asarray(inputs["type_agents"], f32)
    W_self = np.asarray(inputs["W_self"], f32)
    b_self = np.asarray(inputs["b_self"], f32)
    W_merge = np.asarray(inputs["W_merge"], f32)
    b_merge = np.asarray(inputs["b_merge"], f32)
    W_trans = np.asarray(inputs["W_trans"], f32)
    b_trans = np.asarray(inputs["b_trans"], f32)
    W_l = np.asarray(inputs["W_l"], f32)
    W_r = np.asarray(inputs["W_r"], f32)
    w_attn = np.asarray(inputs["w_attn"], f32)
    Wd0 = np.asarray(inputs["Wd0"], f32)
    bd0 = np.asarray(inputs["bd0"], f32)
    Wd1 = np.asarray(inputs["Wd1"], f32)
    bd1 = np.asarray(inputs["bd1"], f32)
    Wd2 = np.asarray(inputs["Wd2"], f32)
    bd2 = np.asarray(inputs["bd2"], f32)

    base = np.zeros((128, CB), f32)
    top = base[0:64]
    bot = base[64:128]
    top[:, C_WL:C_WL + 256] = W_l.T
    top[:, C_WR:C_WR + 256] = W_r.T
    top[:, C_HT + 1:C_HT + N] = ambiguous.T
    bot[:, C_WT:C_WT + 256] = np.concatenate(
        [W_trans[t].T for t in range(RT)], axis=1) / APT
    bot[:, C_WSELF:C_WSELF + H] = W_self.T
    bot[:, C_WML:C_WML + H] = W_merge[:, :H].T
    bot[:, C_WMR:C_WMR + H] = W_merge[:, H:].T
    bot[:, C_WD0B:C_WD0B + H] = Wd0[:, H:].T
    bot[:, C_WD0A:C_WD0A + H] = Wd0[:, :H].T
    bot[:, C_WD1:C_WD1 + SHARD] = Wd1.T
    bot[:, C_TA:C_TA + RT * APT] = type_agents.reshape(RT * APT, H).T
    bot[:, C_BTT:C_BTT + RT] = b_trans.T
    bot[:, C_HID:C_HID + 1] = hidden.T
    bot[:, C_BSC] = b_self
    bot[:, C_BMC] = b_merge
    wexp = np.zeros((128, 128), f32)
    for hh in range(2):
        wexp[hh * 64:(hh + 1) * 64, hh * 64:(hh + 1) * 64] = w_attn[:, None]
    base[:, C_WEXP:C_WEXP + 128] = wexp
    base[:, C_WD2:C_WD2 + RT] = Wd2.T
    base[:, C_BD1] = bd1
    top[:, C_BD0] = bd0
    base[0:RT, C_BD2] = bd2

    amb_pad = np.zeros((H, NCORES * SHARD), f32)
    amb_pad[:, :N_AMB] = ambiguous.T
    in_maps = []
    for cidx in range(NCORES):
        blob = base.copy()
        blob[64:128, C_MLP:C_MLP + SHARD] = \
            amb_pad[:, cidx * SHARD:(cidx + 1) * SHARD]
        in_maps.append({"bfb": blob.astype(bf16)})
    return in_maps


def kernel(**inputs) -> np.ndarray:
    global _compiled
    if _compiled is None:
        _compiled = _build()
    nc = _compiled
    from concourse import bass_utils

    in_maps = _prep_inputs(inputs)
    res = bass_utils.run_bass_kernel_spmd(nc, in_maps, core_ids=list(range(NCORES)))
    out = np.empty((N_AMB, RT), np.float32)
    for cidx in range(NCORES):
        lo = cidx * SHARD
        hi = min(lo + SHARD, N_AMB)
        out[lo:hi, :] = res.results[cidx]["outT"][:, :hi - lo].T
    return out
